# revision 37
# baseline (speedup 1.0000x reference)
import sys
sys.path.insert(0, '/opt/trn_rl_repo')
import numpy as np
import ml_dtypes

import concourse.bass as bass
import concourse.tile as tile
from concourse import bacc, mybir
from concourse.bass_utils import run_bass_kernel_spmd
from concourse.masks import make_identity

DIM = 2048
BSZ, SEQ = 2, 2048
S = SEQ
THRESHOLD = 0.05
HPC = 8                      # q heads per core
KVPC = 2                     # kv heads per core
NPAIR = 4                    # q-head pairs per core
SB = 512
NSB = S // SB                # 4
NDC = DIM // 128             # 16 contraction chunks
NQT = S // 128               # 16 q tiles

f32 = mybir.dt.float32
f32r = mybir.dt.float32r
bf16 = mybir.dt.bfloat16
bf = ml_dtypes.bfloat16
EXP = mybir.ActivationFunctionType.Exp
AX = mybir.AxisListType.X
MAXOP = mybir.AluOpType.max
MINOP = mybir.AluOpType.min
ADDOP = mybir.AluOpType.add

ROW_LAG = 2
USE_DMA_T = False                  # rows between scores and transpose/PV consumption


def _ternarize(w):
    w = w.astype(np.float64)
    scale = max(np.abs(w).mean(), 1e-6)
    return np.where(w > THRESHOLD * scale, 1.0,
                    np.where(w < -THRESHOLD * scale, -1.0, 0.0))


def build_program():
    nc = bacc.Bacc(None, target_bir_lowering=False, debug=False)

    def din(name, shape, dt):
        return nc.dram_tensor(name, list(shape), dt, kind="ExternalInput").ap()

    xT_d = din("xT", (DIM, S), f32r)         # x[b].T fp32
    wq_d = din("wq", (DIM, 512), f32r)       # ternary(wq).T/8 cols (8 heads)
    wk_d = din("wk", (DIM, 128), f32r)
    wv_d = din("wv", (DIM, 128), f32r)
    wo_d = din("wo", (512, DIM), bf16)       # ternary(wo).T rows = core's feats
    tri_d = din("tri", (128, 128), f32)      # strictly-upper -1e30, else 0
    oT_d = nc.dram_tensor("oT", [DIM, S], bf16, kind="ExternalOutput").ap()

    with tile.TileContext(nc) as tc:
        with tc.tile_pool(name="persist", bufs=1) as pp, \
             tc.tile_pool(name="wts", bufs=1) as wp, \
             tc.tile_pool(name="xq", bufs=3) as xqp, \
             tc.tile_pool(name="vfp", bufs=2) as vfp, \
             tc.tile_pool(name="ptp", bufs=3) as ptp, \
             tc.tile_pool(name="ptTp", bufs=2) as ptTp, \
             tc.tile_pool(name="stp", bufs=6) as stp, \
             tc.tile_pool(name="otp", bufs=2) as otpool, \
             tc.tile_pool(name="nop", bufs=2) as nopool, \
             tc.tile_pool(name="obp", bufs=1) as obp, \
             tc.tile_pool(name="acc", bufs=3, space="PSUM") as accp, \
             tc.tile_pool(name="sps", bufs=5, space="PSUM") as sps:

            tri = pp.tile([128, 128], f32)
            nc.sync.dma_start(tri[:], tri_d[:])
            identb = pp.tile([128, 128], bf16)
            make_identity(nc, identb[:])
            # trz: cols 0:512 zeros, 512:640 = tri; slice [640-kw:640] puts the
            # causal mask on the last 128 cols of a kw-wide window.
            trz = pp.tile([128, 640], f32)
            nc.vector.memset(trz[:, 0:512], 0.0)
            nc.vector.tensor_copy(trz[:, 512:640], tri[:])

            qt = [pp.tile([128, S], f32r, tag=f"qt{m}", name=f"qt{m}") for m in range(NPAIR)]
            kk = [pp.tile([128, S], f32r, tag=f"kk{v}", name=f"kk{v}") for v in range(KVPC)]
            va = pp.tile([128, NDC, KVPC, 65], bf16)
            nc.vector.memset(va[:, :, :, 64:65], 1.0)

            # weights resident all run
            wq_f = wp.tile([128, NDC, 512], f32r)
            wk_f = wp.tile([128, NDC, 128], f32r)
            wv_f = wp.tile([128, NDC, 128], f32r)
            wo_b = wp.tile([128, 4, DIM], bf16)
            for g in range(4):
                gs = bass.ds(g * 512, 512)
                qs = bass.ds(g * 4, 4)
                nc.sync.dma_start(
                    wq_f[:, qs, :],
                    wq_d[gs, :].rearrange("(a p) b -> p a b", p=128))
                nc.sync.dma_start(
                    wk_f[:, qs, :],
                    wk_d[gs, :].rearrange("(a p) b -> p a b", p=128))
                nc.sync.dma_start(
                    wv_f[:, qs, :],
                    wv_d[gs, :].rearrange("(a p) b -> p a b", p=128))
            nc.sync.dma_start(
                wo_b[:], wo_d[:, :].rearrange("(a p) b -> p a b", p=128))

            # ---------- emission helpers ----------
            def emit_x_dma(sb_i, g):
                """load dc quad g (4 chunks) of x for seq-block sb_i"""
                xt = xqp.tile([128, 4, SB], f32r, tag="x", name="xt")
                nc.sync.dma_start(
                    xt[:],
                    xT_d[g * 512:(g + 1) * 512, bass.ts(sb_i, SB)].rearrange(
                        "(a p) b -> p a b", p=128))
                return xt

            def emit_proj_pass(sb_i, which):
                """which=0: Q01+K ; which=1: Q23+V."""
                ssl = bass.ts(sb_i, SB)
                a0 = accp.tile([128, SB], f32, tag="acc", name="a0")
                a1 = accp.tile([128, SB], f32, tag="acc", name="a1")
                a2 = accp.tile([128, SB], f32, tag="acc", name="a2")
                xt = None
                for dc in range(NDC):
                    if dc % 4 == 0:
                        xt = emit_x_dma(sb_i, dc // 4)
                    xr = xt[:, dc % 4, :]
                    st = (dc == 0)
                    sp = (dc == NDC - 1)
                    m0, m1 = (0, 1) if which == 0 else (2, 3)
                    nc.tensor.matmul(a0[:], wq_f[:, dc, bass.ts(m0, 128)],
                                     xr, start=st, stop=sp)
                    nc.tensor.matmul(a1[:], wq_f[:, dc, bass.ts(m1, 128)],
                                     xr, start=st, stop=sp)
                    wkv = wk_f if which == 0 else wv_f
                    nc.tensor.matmul(a2[:], wkv[:, dc, :],
                                     xr, start=st, stop=sp)
                # evacuations
                if which == 0:
                    nc.vector.tensor_copy(qt[0][:, ssl], a0[:])
                    nc.scalar.copy(qt[1][:, ssl], a1[:])
                    for v in range(KVPC):
                        nc.vector.tensor_copy(kk[v][0:64, ssl], a2[bass.ds(v * 64, 64), :])
                        nc.scalar.copy(kk[v][64:128, ssl], a2[bass.ds(v * 64, 64), :])
                else:
                    nc.vector.tensor_copy(qt[2][:, ssl], a0[:])
                    nc.scalar.copy(qt[3][:, ssl], a1[:])
                    vf = vfp.tile([128, SB], bf16, tag="vf")
                    nc.scalar.copy(vf[:], a2[:])
                    for j in range(4):
                        c = sb_i * 4 + j
                        vt = accp.tile([128, 128], bf16, tag="acc", name="vt")
                        nc.tensor.matmul(vt[:], vf[:, bass.ts(j, 128)], identb[:],
                                         is_transpose=True, start=True, stop=True)
                        nc.vector.tensor_copy(va[:, c, :, 0:64], vt[:])

            def emit_scores(row):
                """scores + max + exp for one attention row. Returns state."""
                hp, h, qi = row
                kv = hp // 2
                nk = qi // 4 + 1
                qsl = bass.ts(qi, 128)
                lhs_q = qt[hp][bass.ds(h * 64, 64), qsl]
                nmx = stp.tile([128, 4], f32, tag="nmx")
                sblk = []
                for kb in range(nk):
                    kw = 512 if kb < nk - 1 else 128 * (qi % 4 + 1)
                    s0 = sps.tile([128, SB], f32, tag="s", name=f"s{kb}")
                    sblk.append((s0, kw))
                    nc.tensor.matmul(
                        s0[:, 0:kw], lhs_q,
                        kk[kv][bass.ds(h * 64, 64), bass.ds(kb * 512, kw)],
                        start=True, stop=True, tile_position=(h * 64, 0))
                    if kb == nk - 1:
                        nc.vector.tensor_tensor(
                            s0[:, kw - 128:kw], s0[:, kw - 128:kw], tri[:], ADDOP)
                    nc.vector.tensor_reduce(
                        nmx[:, kb:kb + 1], s0[:, 0:kw], AX, MAXOP, negate=True)
                negmax = stp.tile([128, 1], f32, tag="ngm")
                nc.vector.tensor_reduce(negmax[:], nmx[:, 0:nk], AX, MINOP)
                p_t = ptp.tile([128, S], bf16, tag="p")
                for kb, (s0, kw) in enumerate(sblk):
                    nc.scalar.activation(
                        p_t[:, bass.ds(kb * 512, kw)], s0[:, 0:kw],
                        EXP, bias=negmax[:], scale=1.0)
                return p_t

            copy_rr = [0]

            def emit_pv_chunks(pvst, lo, hi):
                """emit PV matmul chunks [lo, hi) for a row's pv state"""
                row, ptT, pvq = pvst
                hp, h, qi = row
                kv = hp // 2
                nch = qi + 1
                for c in range(lo, min(hi, nch)):
                    nc.tensor.matmul(pvq[:], ptT[:, c, :], va[:, c, kv, :],
                                     start=(c == 0), stop=(c == nch - 1),
                                     skip_group_check=True)

            def emit_pv_finish(pvst, nout_t):
                row, ptT, pvq = pvst
                hp, h, qi = row
                rr = stp.tile([128, 1], f32, tag="rr")
                nc.vector.reciprocal(rr[:], pvq[:, 64:65])
                nc.vector.tensor_scalar_mul(nout_t[hp][:, h, qi % 4, :],
                                            pvq[:, 0:64], rr[:])

            def emit_transpose(row, p_t, prev_pvst):
                """transpose P chunks to ptT; weave prev row's PV between
                groups.  Returns (row, ptT, pvq) PV-state for this row."""
                hp, h, qi = row
                nch = qi + 1
                prev_nch = prev_pvst[0][2] + 1 if prev_pvst else 0
                ngroups = (nch + 3) // 4
                pv_per_gap = (prev_nch + ngroups - 1) // ngroups if prev_pvst else 0
                ptT = ptTp.tile([128, NQT, 128], bf16, tag="ptT")
                c = 0
                g = 0
                while c < nch:
                    jn = min(4, nch - c)
                    r = copy_rr[0] % 3
                    copy_rr[0] += 1
                    if r == 2 and USE_DMA_T:
                        # DMA xbar transpose: no PSUM, no copy
                        for j in range(jn):
                            nc.sync.dma_start_transpose(
                                ptT[:, c + j, :], p_t[:, bass.ts(c + j, 128)])
                    else:
                        tp = accp.tile([128, SB], bf16, tag="acc", name="tp")
                        for j in range(jn):
                            nc.tensor.matmul(
                                tp[:, bass.ts(j, 128)],
                                p_t[:, bass.ts(c + j, 128)], identb[:],
                                is_transpose=True, start=(j == 0), stop=(j == jn - 1))
                        dst = ptT[:, c:c + jn, :].rearrange("p a b -> p (a b)")
                        src = tp[:, 0:jn * 128]
                        if r == 0:
                            nc.vector.tensor_copy(dst, src)
                        else:
                            nc.scalar.copy(dst, src)
                    if prev_pvst:
                        emit_pv_chunks(prev_pvst, g * pv_per_gap, (g + 1) * pv_per_gap)
                    c += jn
                    g += 1
                if prev_pvst:
                    emit_pv_chunks(prev_pvst, g * pv_per_gap, prev_nch)
                pvq = accp.tile([128, 65], f32, tag="acc", name="pvq")
                return (row, ptT, pvq)

            def emit_otT(sb_i, nout_t, ot_t):
                for hp in range(NPAIR):
                    for h in range(2):
                        otp = accp.tile([64, SB], bf16, tag="acc", name="otp")
                        for j in range(4):
                            nc.tensor.matmul(
                                otp[:, bass.ts(j, 128)],
                                nout_t[hp][:, h, j, :], identb[:],
                                is_transpose=True, start=(j == 0), stop=(j == 3))
                        nc.vector.tensor_copy(
                            ot_t[hp][bass.ds(h * 64, 64), :], otp[:])

            def emit_oproj_item(sb_i, mo, ot_t, ob):
                pso = accp.tile([128, SB], f32, tag="acc", name="pso")
                for fc in range(4):
                    nc.tensor.matmul(
                        pso[:], wo_b[:, fc, bass.ts(mo, 128)],
                        ot_t[fc][:, :], start=(fc == 0), stop=(fc == 3))
                if mo % 2 == 0:
                    nc.vector.tensor_copy(ob[:, mo, :], pso[:])
                else:
                    nc.scalar.copy(ob[:, mo, :], pso[:])

            def emit_out_dma(sb_i, ob):
                nc.sync.dma_start(
                    oT_d[:, bass.ts(sb_i, SB)].rearrange("(a p) b -> p a b", p=128),
                    ob[:])

            # ---------- main pipelined schedule ----------
            # Global row pipeline carried across seq-blocks; otT/o-proj for a
            # block are emitted as soon as its last row finishes, and o-proj
            # items weave between later rows as PE filler.
            pvst = None           # PV-state: row whose PV chunks go in next gaps
            pending = []          # rows awaiting transpose stage
            oproj_q = []          # pending o-proj filler items
            sb_state = {}         # sb_i -> dict(nout_t, ot_t, left, ob, emitted)

            def row_finished(row):
                hp, h, qi = row
                fsb = qi // 4
                st = sb_state[fsb]
                st["left"] -= 1
                if st["left"] == 0:
                    emit_otT(fsb, st["nout_t"], st["ot_t"])
                    ob = obp.tile([128, 16, SB], bf16, tag="ob", name="ob")
                    st["ob"] = ob
                    for mo in range(16):
                        oproj_q.append((fsb, mo))

            def pump_oproj(n):
                for _ in range(n):
                    if not oproj_q:
                        return
                    fsb, mo = oproj_q.pop(0)
                    st = sb_state[fsb]
                    emit_oproj_item(fsb, mo, st["ot_t"], st["ob"])
                    st["emitted"] += 1
                    if st["emitted"] == 16:
                        emit_out_dma(fsb, st["ob"])

            def pop_pending():
                nonlocal pvst
                prow, pp_t = pending.pop(0)
                pvst_new = emit_transpose(prow, pp_t, pvst)
                if pvst is not None:
                    fr = pvst[0]
                    emit_pv_finish(pvst, sb_state[fr[2] // 4]["nout_t"])
                    row_finished(fr)
                pvst = pvst_new

            for sb_i in range(NSB):
                emit_proj_pass(sb_i, 0)
                emit_proj_pass(sb_i, 1)
                sb_state[sb_i] = {
                    "nout_t": [nopool.tile([128, 2, 4, 64], bf16, tag=f"no{hp}",
                                           name=f"no{hp}") for hp in range(NPAIR)],
                    "ot_t": [otpool.tile([128, SB], bf16, tag=f"ot{hp}",
                                         name=f"ot{hp}") for hp in range(NPAIR)],
                    "left": 32, "ob": None, "emitted": 0,
                }
                rows = [(hp, h, sb_i * 4 + j)
                        for j in range(4) for hp in range(NPAIR) for h in range(2)]
                for ri, row in enumerate(rows):
                    p_t = emit_scores(row)
                    pending.append((row, p_t))
                    if ri % 2 == 1:
                        pump_oproj(1)
                    if len(pending) > ROW_LAG:
                        pop_pending()

            # drain the pipeline
            while pending:
                pop_pending()
            if pvst is not None:
                emit_pv_chunks(pvst, 0, pvst[0][2] + 1)
                fr = pvst[0]
                emit_pv_finish(pvst, sb_state[fr[2] // 4]["nout_t"])
                row_finished(fr)
                pvst = None
            pump_oproj(len(oproj_q) + 16)

    nc.compile()
    return nc


_PROG = None


def kernel(x, wq, wk, wv, wo):
    global _PROG
    if _PROG is None:
        _PROG = build_program()
    nc = _PROG

    twq = _ternarize(wq) / 8.0          # fold softmax scale into q
    twk = _ternarize(wk)
    twv = _ternarize(wv)
    two = _ternarize(wo)
    tri_np = (np.triu(np.ones((128, 128), np.float64), 1) * -1e30).astype(np.float32)

    xT = [np.ascontiguousarray(x[b].astype(np.float32).T) for b in range(BSZ)]
    in_maps = []
    for c in range(8):
        b, hq = c % 2, c // 2
        qcols = slice(hq * 512, (hq + 1) * 512)
        kvcols = slice(hq * 128, (hq + 1) * 128)
        in_maps.append({
            "xT": xT[b],
            "wq": np.ascontiguousarray(twq.T[:, qcols]).astype(np.float32),
            "wk": np.ascontiguousarray(twk.T[:, kvcols]).astype(np.float32),
            "wv": np.ascontiguousarray(twv.T[:, kvcols]).astype(np.float32),
            "wo": np.ascontiguousarray(two.T[hq * 512:(hq + 1) * 512, :]).astype(bf),
            "tri": tri_np,
        })

    res = run_bass_kernel_spmd(nc, in_maps, list(range(8)))

    out = np.zeros((BSZ, SEQ, DIM), np.float32)
    for c in range(8):
        b = c % 2
        out[b] += res.results[c]["oT"].astype(np.float32).T
    return out


# revision 38
# speedup vs baseline: 1.0424x; 1.0424x over previous
import sys
sys.path.insert(0, '/opt/trn_rl_repo')
import numpy as np
import ml_dtypes

import concourse.bass as bass
import concourse.tile as tile
from concourse import bacc, mybir
from concourse.bass_utils import run_bass_kernel_spmd
from concourse.masks import make_identity

DIM = 2048
BSZ, SEQ = 2, 2048
S = SEQ
THRESHOLD = 0.05
HPC = 8                      # q heads per core
KVPC = 2                     # kv heads per core
NPAIR = 4                    # q-head pairs per core
SB = 512
NSB = S // SB                # 4
NDC = DIM // 128             # 16 contraction chunks
NQT = S // 128               # 16 q tiles

f32 = mybir.dt.float32
f32r = mybir.dt.float32r
bf16 = mybir.dt.bfloat16
bf = ml_dtypes.bfloat16
EXP = mybir.ActivationFunctionType.Exp
AX = mybir.AxisListType.X
MAXOP = mybir.AluOpType.max
MINOP = mybir.AluOpType.min
ADDOP = mybir.AluOpType.add

import os
ROW_LAG = int(os.environ.get("K_ROW_LAG", "2"))
USE_DMA_T = os.environ.get("K_DMA_T", "0") == "1"
PTP_BUFS = int(os.environ.get("K_PTP", "3"))
PTT_BUFS = int(os.environ.get("K_PTT", "2"))
COPY_PAT = os.environ.get("K_COPY", "dad")   # per-group engine cycle: d=DVE a=ACT
OPW = int(os.environ.get("K_OPW", "2"))      # oproj weave: 1 item per OPW rows


def _ternarize(w):
    w = w.astype(np.float64)
    scale = max(np.abs(w).mean(), 1e-6)
    return np.where(w > THRESHOLD * scale, 1.0,
                    np.where(w < -THRESHOLD * scale, -1.0, 0.0))


def build_program():
    nc = bacc.Bacc(None, target_bir_lowering=False, debug=False)

    def din(name, shape, dt):
        return nc.dram_tensor(name, list(shape), dt, kind="ExternalInput").ap()

    xT_d = din("xT", (DIM, S), f32r)         # x[b].T fp32
    wq_d = din("wq", (DIM, 512), f32r)       # ternary(wq).T/8 cols (8 heads)
    wk_d = din("wk", (DIM, 128), f32r)
    wv_d = din("wv", (DIM, 128), f32r)
    wo_d = din("wo", (512, DIM), bf16)       # ternary(wo).T rows = core's feats
    tri_d = din("tri", (128, 128), f32)      # strictly-upper -1e30, else 0
    oT_d = nc.dram_tensor("oT", [DIM, S], bf16, kind="ExternalOutput").ap()

    with tile.TileContext(nc) as tc:
        with tc.tile_pool(name="persist", bufs=1) as pp, \
             tc.tile_pool(name="wts", bufs=1) as wp, \
             tc.tile_pool(name="xq", bufs=3) as xqp, \
             tc.tile_pool(name="vfp", bufs=2) as vfp, \
             tc.tile_pool(name="ptp", bufs=PTP_BUFS) as ptp, \
             tc.tile_pool(name="ptTp", bufs=PTT_BUFS) as ptTp, \
             tc.tile_pool(name="stp", bufs=6) as stp, \
             tc.tile_pool(name="otp", bufs=2) as otpool, \
             tc.tile_pool(name="nop", bufs=2) as nopool, \
             tc.tile_pool(name="obp", bufs=1) as obp, \
             tc.tile_pool(name="acc", bufs=3, space="PSUM") as accp, \
             tc.tile_pool(name="sps", bufs=5, space="PSUM") as sps:

            tri = pp.tile([128, 128], f32)
            nc.sync.dma_start(tri[:], tri_d[:])
            identb = pp.tile([128, 128], bf16)
            make_identity(nc, identb[:])
            # trz: cols 0:512 zeros, 512:640 = tri; slice [640-kw:640] puts the
            # causal mask on the last 128 cols of a kw-wide window.
            trz = pp.tile([128, 640], f32)
            nc.vector.memset(trz[:, 0:512], 0.0)
            nc.vector.tensor_copy(trz[:, 512:640], tri[:])

            qt = [pp.tile([128, S], f32r, tag=f"qt{m}", name=f"qt{m}") for m in range(NPAIR)]
            kk = [pp.tile([128, S], f32r, tag=f"kk{v}", name=f"kk{v}") for v in range(KVPC)]
            va = pp.tile([128, NDC, KVPC, 65], bf16)
            nc.vector.memset(va[:, :, :, 64:65], 1.0)

            # weights resident all run
            wq_f = wp.tile([128, NDC, 512], f32r)
            wk_f = wp.tile([128, NDC, 128], f32r)
            wv_f = wp.tile([128, NDC, 128], f32r)
            wo_b = wp.tile([128, 4, DIM], bf16)
            for g in range(4):
                gs = bass.ds(g * 512, 512)
                qs = bass.ds(g * 4, 4)
                nc.sync.dma_start(
                    wq_f[:, qs, :],
                    wq_d[gs, :].rearrange("(a p) b -> p a b", p=128))
                nc.sync.dma_start(
                    wk_f[:, qs, :],
                    wk_d[gs, :].rearrange("(a p) b -> p a b", p=128))
                nc.sync.dma_start(
                    wv_f[:, qs, :],
                    wv_d[gs, :].rearrange("(a p) b -> p a b", p=128))
            nc.sync.dma_start(
                wo_b[:], wo_d[:, :].rearrange("(a p) b -> p a b", p=128))

            # ---------- emission helpers ----------
            def emit_x_dma(sb_i, g):
                """load dc quad g (4 chunks) of x for seq-block sb_i"""
                xt = xqp.tile([128, 4, SB], f32r, tag="x", name="xt")
                nc.sync.dma_start(
                    xt[:],
                    xT_d[g * 512:(g + 1) * 512, bass.ts(sb_i, SB)].rearrange(
                        "(a p) b -> p a b", p=128))
                return xt

            def emit_proj_pass(sb_i, which):
                """which=0: Q01+K ; which=1: Q23+V."""
                ssl = bass.ts(sb_i, SB)
                a0 = accp.tile([128, SB], f32, tag="acc", name="a0")
                a1 = accp.tile([128, SB], f32, tag="acc", name="a1")
                a2 = accp.tile([128, SB], f32, tag="acc", name="a2")
                xt = None
                for dc in range(NDC):
                    if dc % 4 == 0:
                        xt = emit_x_dma(sb_i, dc // 4)
                    xr = xt[:, dc % 4, :]
                    st = (dc == 0)
                    sp = (dc == NDC - 1)
                    m0, m1 = (0, 1) if which == 0 else (2, 3)
                    nc.tensor.matmul(a0[:], wq_f[:, dc, bass.ts(m0, 128)],
                                     xr, start=st, stop=sp)
                    nc.tensor.matmul(a1[:], wq_f[:, dc, bass.ts(m1, 128)],
                                     xr, start=st, stop=sp)
                    wkv = wk_f if which == 0 else wv_f
                    nc.tensor.matmul(a2[:], wkv[:, dc, :],
                                     xr, start=st, stop=sp)
                # evacuations
                if which == 0:
                    nc.vector.tensor_copy(qt[0][:, ssl], a0[:])
                    nc.scalar.copy(qt[1][:, ssl], a1[:])
                    for v in range(KVPC):
                        nc.vector.tensor_copy(kk[v][0:64, ssl], a2[bass.ds(v * 64, 64), :])
                        nc.scalar.copy(kk[v][64:128, ssl], a2[bass.ds(v * 64, 64), :])
                else:
                    nc.vector.tensor_copy(qt[2][:, ssl], a0[:])
                    nc.scalar.copy(qt[3][:, ssl], a1[:])
                    vf = vfp.tile([128, SB], bf16, tag="vf")
                    nc.scalar.copy(vf[:], a2[:])
                    for j in range(4):
                        c = sb_i * 4 + j
                        vt = accp.tile([128, 128], bf16, tag="acc", name="vt")
                        nc.tensor.matmul(vt[:], vf[:, bass.ts(j, 128)], identb[:],
                                         is_transpose=True, start=True, stop=True)
                        nc.vector.tensor_copy(va[:, c, :, 0:64], vt[:])

            def emit_scores(row):
                """scores + max + exp for one attention row. Returns state."""
                hp, h, qi = row
                kv = hp // 2
                nk = qi // 4 + 1
                qsl = bass.ts(qi, 128)
                lhs_q = qt[hp][bass.ds(h * 64, 64), qsl]
                nmx = stp.tile([128, 4], f32, tag="nmx")
                sblk = []
                for kb in range(nk):
                    kw = 512 if kb < nk - 1 else 128 * (qi % 4 + 1)
                    s0 = sps.tile([128, SB], f32, tag="s", name=f"s{kb}")
                    sblk.append((s0, kw))
                    nc.tensor.matmul(
                        s0[:, 0:kw], lhs_q,
                        kk[kv][bass.ds(h * 64, 64), bass.ds(kb * 512, kw)],
                        start=True, stop=True, tile_position=(h * 64, 0))
                    if kb == nk - 1:
                        nc.vector.tensor_tensor(
                            s0[:, kw - 128:kw], s0[:, kw - 128:kw], tri[:], ADDOP)
                    nc.vector.tensor_reduce(
                        nmx[:, kb:kb + 1], s0[:, 0:kw], AX, MAXOP, negate=True)
                negmax = stp.tile([128, 1], f32, tag="ngm")
                nc.vector.tensor_reduce(negmax[:], nmx[:, 0:nk], AX, MINOP)
                p_t = ptp.tile([128, S], bf16, tag="p")
                for kb, (s0, kw) in enumerate(sblk):
                    nc.scalar.activation(
                        p_t[:, bass.ds(kb * 512, kw)], s0[:, 0:kw],
                        EXP, bias=negmax[:], scale=1.0)
                return p_t

            copy_rr = [0]

            def emit_pv_chunks(pvst, lo, hi):
                """emit PV matmul chunks [lo, hi) for a row's pv state"""
                row, ptT, pvq = pvst
                hp, h, qi = row
                kv = hp // 2
                nch = qi + 1
                for c in range(lo, min(hi, nch)):
                    nc.tensor.matmul(pvq[:], ptT[:, c, :], va[:, c, kv, :],
                                     start=(c == 0), stop=(c == nch - 1),
                                     skip_group_check=True)

            def emit_pv_finish(pvst, nout_t):
                row, ptT, pvq = pvst
                hp, h, qi = row
                rr = stp.tile([128, 1], f32, tag="rr")
                nc.vector.reciprocal(rr[:], pvq[:, 64:65])
                nc.vector.tensor_scalar_mul(nout_t[hp][:, h, qi % 4, :],
                                            pvq[:, 0:64], rr[:])

            def emit_transpose(row, p_t, prev_pvst):
                """transpose P chunks to ptT; weave prev row's PV between
                groups.  Returns (row, ptT, pvq) PV-state for this row."""
                hp, h, qi = row
                nch = qi + 1
                prev_nch = prev_pvst[0][2] + 1 if prev_pvst else 0
                ngroups = (nch + 3) // 4
                pv_per_gap = (prev_nch + ngroups - 1) // ngroups if prev_pvst else 0
                ptT = ptTp.tile([128, NQT, 128], bf16, tag="ptT")
                c = 0
                g = 0
                while c < nch:
                    jn = min(4, nch - c)
                    r = copy_rr[0] % len(COPY_PAT)
                    ce = COPY_PAT[r]
                    copy_rr[0] += 1
                    if ce == "x" or (USE_DMA_T and ce == "m"):
                        # DMA xbar transpose: no PSUM, no copy
                        for j in range(jn):
                            nc.sync.dma_start_transpose(
                                ptT[:, c + j, :], p_t[:, bass.ts(c + j, 128)])
                    else:
                        tp = accp.tile([128, SB], bf16, tag="acc", name="tp")
                        for j in range(jn):
                            nc.tensor.matmul(
                                tp[:, bass.ts(j, 128)],
                                p_t[:, bass.ts(c + j, 128)], identb[:],
                                is_transpose=True, start=(j == 0), stop=(j == jn - 1))
                        dst = ptT[:, c:c + jn, :].rearrange("p a b -> p (a b)")
                        src = tp[:, 0:jn * 128]
                        if ce in ("d", "m"):
                            nc.vector.tensor_copy(dst, src)
                        else:
                            nc.scalar.copy(dst, src)
                    if prev_pvst:
                        emit_pv_chunks(prev_pvst, g * pv_per_gap, (g + 1) * pv_per_gap)
                    c += jn
                    g += 1
                if prev_pvst:
                    emit_pv_chunks(prev_pvst, g * pv_per_gap, prev_nch)
                pvq = accp.tile([128, 65], f32, tag="acc", name="pvq")
                return (row, ptT, pvq)

            def emit_otT(sb_i, nout_t, ot_t):
                for hp in range(NPAIR):
                    for h in range(2):
                        otp = accp.tile([64, SB], bf16, tag="acc", name="otp")
                        for j in range(4):
                            nc.tensor.matmul(
                                otp[:, bass.ts(j, 128)],
                                nout_t[hp][:, h, j, :], identb[:],
                                is_transpose=True, start=(j == 0), stop=(j == 3))
                        nc.vector.tensor_copy(
                            ot_t[hp][bass.ds(h * 64, 64), :], otp[:])

            def emit_oproj_item(sb_i, mo, ot_t, ob):
                pso = accp.tile([128, SB], f32, tag="acc", name="pso")
                for fc in range(4):
                    nc.tensor.matmul(
                        pso[:], wo_b[:, fc, bass.ts(mo, 128)],
                        ot_t[fc][:, :], start=(fc == 0), stop=(fc == 3))
                if mo % 2 == 0:
                    nc.vector.tensor_copy(ob[:, mo, :], pso[:])
                else:
                    nc.scalar.copy(ob[:, mo, :], pso[:])

            def emit_out_dma(sb_i, ob):
                nc.sync.dma_start(
                    oT_d[:, bass.ts(sb_i, SB)].rearrange("(a p) b -> p a b", p=128),
                    ob[:])

            # ---------- main pipelined schedule ----------
            # Global row pipeline carried across seq-blocks; otT/o-proj for a
            # block are emitted as soon as its last row finishes, and o-proj
            # items weave between later rows as PE filler.
            pvst = None           # PV-state: row whose PV chunks go in next gaps
            pending = []          # rows awaiting transpose stage
            oproj_q = []          # pending o-proj filler items
            sb_state = {}         # sb_i -> dict(nout_t, ot_t, left, ob, emitted)

            def row_finished(row):
                hp, h, qi = row
                fsb = qi // 4
                st = sb_state[fsb]
                st["left"] -= 1
                if st["left"] == 0:
                    emit_otT(fsb, st["nout_t"], st["ot_t"])
                    ob = obp.tile([128, 16, SB], bf16, tag="ob", name="ob")
                    st["ob"] = ob
                    for mo in range(16):
                        oproj_q.append((fsb, mo))

            def pump_oproj(n):
                for _ in range(n):
                    if not oproj_q:
                        return
                    fsb, mo = oproj_q.pop(0)
                    st = sb_state[fsb]
                    emit_oproj_item(fsb, mo, st["ot_t"], st["ob"])
                    st["emitted"] += 1
                    if st["emitted"] == 16:
                        emit_out_dma(fsb, st["ob"])

            def pop_pending():
                nonlocal pvst
                prow, pp_t = pending.pop(0)
                pvst_new = emit_transpose(prow, pp_t, pvst)
                if pvst is not None:
                    fr = pvst[0]
                    emit_pv_finish(pvst, sb_state[fr[2] // 4]["nout_t"])
                    row_finished(fr)
                pvst = pvst_new

            for sb_i in range(NSB):
                emit_proj_pass(sb_i, 0)
                emit_proj_pass(sb_i, 1)
                sb_state[sb_i] = {
                    "nout_t": [nopool.tile([128, 2, 4, 64], bf16, tag=f"no{hp}",
                                           name=f"no{hp}") for hp in range(NPAIR)],
                    "ot_t": [otpool.tile([128, SB], bf16, tag=f"ot{hp}",
                                         name=f"ot{hp}") for hp in range(NPAIR)],
                    "left": 32, "ob": None, "emitted": 0,
                }
                rows = [(hp, h, sb_i * 4 + j)
                        for j in range(4) for hp in range(NPAIR) for h in range(2)]
                for ri, row in enumerate(rows):
                    p_t = emit_scores(row)
                    pending.append((row, p_t))
                    if ri % OPW == OPW - 1:
                        pump_oproj(1)
                    if len(pending) > ROW_LAG:
                        pop_pending()

            # drain the pipeline
            while pending:
                pop_pending()
            if pvst is not None:
                emit_pv_chunks(pvst, 0, pvst[0][2] + 1)
                fr = pvst[0]
                emit_pv_finish(pvst, sb_state[fr[2] // 4]["nout_t"])
                row_finished(fr)
                pvst = None
            pump_oproj(len(oproj_q) + 16)

    nc.compile()
    return nc


_PROG = None


def kernel(x, wq, wk, wv, wo):
    global _PROG
    if _PROG is None:
        _PROG = build_program()
    nc = _PROG

    twq = _ternarize(wq) / 8.0          # fold softmax scale into q
    twk = _ternarize(wk)
    twv = _ternarize(wv)
    two = _ternarize(wo)
    tri_np = (np.triu(np.ones((128, 128), np.float64), 1) * -1e30).astype(np.float32)

    xT = [np.ascontiguousarray(x[b].astype(np.float32).T) for b in range(BSZ)]
    in_maps = []
    for c in range(8):
        b, hq = c % 2, c // 2
        qcols = slice(hq * 512, (hq + 1) * 512)
        kvcols = slice(hq * 128, (hq + 1) * 128)
        in_maps.append({
            "xT": xT[b],
            "wq": np.ascontiguousarray(twq.T[:, qcols]).astype(np.float32),
            "wk": np.ascontiguousarray(twk.T[:, kvcols]).astype(np.float32),
            "wv": np.ascontiguousarray(twv.T[:, kvcols]).astype(np.float32),
            "wo": np.ascontiguousarray(two.T[hq * 512:(hq + 1) * 512, :]).astype(bf),
            "tri": tri_np,
        })

    res = run_bass_kernel_spmd(nc, in_maps, list(range(8)))

    out = np.zeros((BSZ, SEQ, DIM), np.float32)
    for c in range(8):
        b = c % 2
        out[b] += res.results[c]["oT"].astype(np.float32).T
    return out


# revision 39
# speedup vs baseline: 1.0460x; 1.0034x over previous
import sys
sys.path.insert(0, '/opt/trn_rl_repo')
import numpy as np
import ml_dtypes

import concourse.bass as bass
import concourse.tile as tile
from concourse import bacc, mybir
from concourse.bass_utils import run_bass_kernel_spmd
from concourse.masks import make_identity

DIM = 2048
BSZ, SEQ = 2, 2048
S = SEQ
THRESHOLD = 0.05
HPC = 8                      # q heads per core
KVPC = 2                     # kv heads per core
NPAIR = 4                    # q-head pairs per core
SB = 512
NSB = S // SB                # 4
NDC = DIM // 128             # 16 contraction chunks
NQT = S // 128               # 16 q tiles

f32 = mybir.dt.float32
f32r = mybir.dt.float32r
bf16 = mybir.dt.bfloat16
bf = ml_dtypes.bfloat16
EXP = mybir.ActivationFunctionType.Exp
AX = mybir.AxisListType.X
MAXOP = mybir.AluOpType.max
MINOP = mybir.AluOpType.min
ADDOP = mybir.AluOpType.add

import os
ROW_LAG = int(os.environ.get("K_ROW_LAG", "2"))
USE_DMA_T = os.environ.get("K_DMA_T", "0") == "1"
PTP_BUFS = int(os.environ.get("K_PTP", "3"))
PTT_BUFS = int(os.environ.get("K_PTT", "2"))
COPY_PAT = os.environ.get("K_COPY", "dad")   # per-group engine cycle: d=DVE a=ACT
OPW = int(os.environ.get("K_OPW", "2"))      # oproj weave: 1 item per OPW rows


def _ternarize(w):
    w = w.astype(np.float64)
    scale = max(np.abs(w).mean(), 1e-6)
    return np.where(w > THRESHOLD * scale, 1.0,
                    np.where(w < -THRESHOLD * scale, -1.0, 0.0))


def build_program():
    nc = bacc.Bacc(None, target_bir_lowering=False, debug=False)

    def din(name, shape, dt):
        return nc.dram_tensor(name, list(shape), dt, kind="ExternalInput").ap()

    xT_d = din("xT", (DIM, S), f32r)         # x[b].T fp32
    wq_d = din("wq", (DIM, 512), f32r)       # ternary(wq).T/8 cols (8 heads)
    wk_d = din("wk", (DIM, 128), f32r)
    wv_d = din("wv", (DIM, 128), f32r)
    wo_d = din("wo", (512, DIM), bf16)       # ternary(wo).T rows = core's feats
    tri_d = din("tri", (128, 128), f32)      # strictly-upper -1e30, else 0
    oT_d = nc.dram_tensor("oT", [DIM, S], bf16, kind="ExternalOutput").ap()

    with tile.TileContext(nc) as tc:
        with tc.tile_pool(name="persist", bufs=1) as pp, \
             tc.tile_pool(name="wts", bufs=1) as wp, \
             tc.tile_pool(name="xq", bufs=3) as xqp, \
             tc.tile_pool(name="vfp", bufs=2) as vfp, \
             tc.tile_pool(name="ptp", bufs=PTP_BUFS) as ptp, \
             tc.tile_pool(name="ptTp", bufs=PTT_BUFS) as ptTp, \
             tc.tile_pool(name="stp", bufs=6) as stp, \
             tc.tile_pool(name="otp", bufs=2) as otpool, \
             tc.tile_pool(name="nop", bufs=2) as nopool, \
             tc.tile_pool(name="obp", bufs=1) as obp, \
             tc.tile_pool(name="acc", bufs=3, space="PSUM") as accp, \
             tc.tile_pool(name="sps", bufs=5, space="PSUM") as sps:

            tri = pp.tile([128, 128], f32)
            nc.sync.dma_start(tri[:], tri_d[:])
            identb = pp.tile([128, 128], bf16)
            make_identity(nc, identb[:])
            # trz: cols 0:512 zeros, 512:640 = tri; slice [640-kw:640] puts the
            # causal mask on the last 128 cols of a kw-wide window.
            trz = pp.tile([128, 640], f32)
            nc.vector.memset(trz[:, 0:512], 0.0)
            nc.vector.tensor_copy(trz[:, 512:640], tri[:])

            qt = [pp.tile([128, S], f32r, tag=f"qt{m}", name=f"qt{m}") for m in range(NPAIR)]
            kk = [pp.tile([128, S], f32r, tag=f"kk{v}", name=f"kk{v}") for v in range(KVPC)]
            va = pp.tile([128, NDC, KVPC, 65], bf16)
            nc.vector.memset(va[:, :, :, 64:65], 1.0)

            # weights resident all run
            wq_f = wp.tile([128, NDC, 512], f32r)
            wk_f = wp.tile([128, NDC, 128], f32r)
            wv_f = wp.tile([128, NDC, 128], f32r)
            wo_b = wp.tile([128, 4, DIM], bf16)
            wstate = {"g": 0, "wo": False}

            def emit_w_dma_chunk():
                g = wstate["g"]
                if g < 4:
                    gs = bass.ds(g * 512, 512)
                    qs = bass.ds(g * 4, 4)
                    nc.sync.dma_start(
                        wq_f[:, qs, :],
                        wq_d[gs, :].rearrange("(a p) b -> p a b", p=128))
                    nc.sync.dma_start(
                        wk_f[:, qs, :],
                        wk_d[gs, :].rearrange("(a p) b -> p a b", p=128))
                    nc.sync.dma_start(
                        wv_f[:, qs, :],
                        wv_d[gs, :].rearrange("(a p) b -> p a b", p=128))
                    wstate["g"] = g + 1
                elif not wstate["wo"]:
                    nc.sync.dma_start(
                        wo_b[:], wo_d[:, :].rearrange("(a p) b -> p a b", p=128))
                    wstate["wo"] = True

            # ---------- emission helpers ----------
            def emit_x_dma(sb_i, g):
                """load dc quad g (4 chunks) of x for seq-block sb_i"""
                xt = xqp.tile([128, 4, SB], f32r, tag="x", name="xt")
                nc.sync.dma_start(
                    xt[:],
                    xT_d[g * 512:(g + 1) * 512, bass.ts(sb_i, SB)].rearrange(
                        "(a p) b -> p a b", p=128))
                return xt

            def emit_proj_pass(sb_i, which):
                """which=0: Q01+K ; which=1: Q23+V."""
                ssl = bass.ts(sb_i, SB)
                a0 = accp.tile([128, SB], f32, tag="acc", name="a0")
                a1 = accp.tile([128, SB], f32, tag="acc", name="a1")
                a2 = accp.tile([128, SB], f32, tag="acc", name="a2")
                xt = None
                for dc in range(NDC):
                    if dc % 4 == 0:
                        xt = emit_x_dma(sb_i, dc // 4)
                        emit_w_dma_chunk()
                    xr = xt[:, dc % 4, :]
                    st = (dc == 0)
                    sp = (dc == NDC - 1)
                    m0, m1 = (0, 1) if which == 0 else (2, 3)
                    nc.tensor.matmul(a0[:], wq_f[:, dc, bass.ts(m0, 128)],
                                     xr, start=st, stop=sp)
                    nc.tensor.matmul(a1[:], wq_f[:, dc, bass.ts(m1, 128)],
                                     xr, start=st, stop=sp)
                    wkv = wk_f if which == 0 else wv_f
                    nc.tensor.matmul(a2[:], wkv[:, dc, :],
                                     xr, start=st, stop=sp)
                # evacuations
                if which == 0:
                    nc.vector.tensor_copy(qt[0][:, ssl], a0[:])
                    nc.scalar.copy(qt[1][:, ssl], a1[:])
                    for v in range(KVPC):
                        nc.vector.tensor_copy(kk[v][0:64, ssl], a2[bass.ds(v * 64, 64), :])
                        nc.scalar.copy(kk[v][64:128, ssl], a2[bass.ds(v * 64, 64), :])
                else:
                    nc.vector.tensor_copy(qt[2][:, ssl], a0[:])
                    nc.scalar.copy(qt[3][:, ssl], a1[:])
                    vf = vfp.tile([128, SB], bf16, tag="vf")
                    nc.scalar.copy(vf[:], a2[:])
                    for j in range(4):
                        c = sb_i * 4 + j
                        vt = accp.tile([128, 128], bf16, tag="acc", name="vt")
                        nc.tensor.matmul(vt[:], vf[:, bass.ts(j, 128)], identb[:],
                                         is_transpose=True, start=True, stop=True)
                        nc.vector.tensor_copy(va[:, c, :, 0:64], vt[:])

            def emit_scores(row):
                """scores + max + exp for one attention row. Returns state."""
                hp, h, qi = row
                kv = hp // 2
                nk = qi // 4 + 1
                qsl = bass.ts(qi, 128)
                lhs_q = qt[hp][bass.ds(h * 64, 64), qsl]
                nmx = stp.tile([128, 4], f32, tag="nmx")
                sblk = []
                for kb in range(nk):
                    kw = 512 if kb < nk - 1 else 128 * (qi % 4 + 1)
                    s0 = sps.tile([128, SB], f32, tag="s", name=f"s{kb}")
                    sblk.append((s0, kw))
                    nc.tensor.matmul(
                        s0[:, 0:kw], lhs_q,
                        kk[kv][bass.ds(h * 64, 64), bass.ds(kb * 512, kw)],
                        start=True, stop=True, tile_position=(h * 64, 0))
                    if kb == nk - 1:
                        nc.vector.tensor_tensor(
                            s0[:, kw - 128:kw], s0[:, kw - 128:kw], tri[:], ADDOP)
                    nc.vector.tensor_reduce(
                        nmx[:, kb:kb + 1], s0[:, 0:kw], AX, MAXOP, negate=True)
                negmax = stp.tile([128, 1], f32, tag="ngm")
                nc.vector.tensor_reduce(negmax[:], nmx[:, 0:nk], AX, MINOP)
                p_t = ptp.tile([128, S], bf16, tag="p")
                for kb, (s0, kw) in enumerate(sblk):
                    nc.scalar.activation(
                        p_t[:, bass.ds(kb * 512, kw)], s0[:, 0:kw],
                        EXP, bias=negmax[:], scale=1.0)
                return p_t

            copy_rr = [0]

            def emit_pv_chunks(pvst, lo, hi):
                """emit PV matmul chunks [lo, hi) for a row's pv state"""
                row, ptT, pvq = pvst
                hp, h, qi = row
                kv = hp // 2
                nch = qi + 1
                for c in range(lo, min(hi, nch)):
                    nc.tensor.matmul(pvq[:], ptT[:, c, :], va[:, c, kv, :],
                                     start=(c == 0), stop=(c == nch - 1),
                                     skip_group_check=True)

            def emit_pv_finish(pvst, nout_t):
                row, ptT, pvq = pvst
                hp, h, qi = row
                rr = stp.tile([128, 1], f32, tag="rr")
                nc.vector.reciprocal(rr[:], pvq[:, 64:65])
                nc.vector.tensor_scalar_mul(nout_t[hp][:, h, qi % 4, :],
                                            pvq[:, 0:64], rr[:])

            def emit_transpose(row, p_t, prev_pvst):
                """transpose P chunks to ptT; weave prev row's PV between
                groups.  Returns (row, ptT, pvq) PV-state for this row."""
                hp, h, qi = row
                nch = qi + 1
                prev_nch = prev_pvst[0][2] + 1 if prev_pvst else 0
                ngroups = (nch + 3) // 4
                pv_per_gap = (prev_nch + ngroups - 1) // ngroups if prev_pvst else 0
                ptT = ptTp.tile([128, NQT, 128], bf16, tag="ptT")
                c = 0
                g = 0
                while c < nch:
                    jn = min(4, nch - c)
                    r = copy_rr[0] % len(COPY_PAT)
                    ce = COPY_PAT[r]
                    copy_rr[0] += 1
                    if ce == "x" or (USE_DMA_T and ce == "m"):
                        # DMA xbar transpose: no PSUM, no copy
                        for j in range(jn):
                            nc.sync.dma_start_transpose(
                                ptT[:, c + j, :], p_t[:, bass.ts(c + j, 128)])
                    else:
                        tp = accp.tile([128, SB], bf16, tag="acc", name="tp")
                        for j in range(jn):
                            nc.tensor.matmul(
                                tp[:, bass.ts(j, 128)],
                                p_t[:, bass.ts(c + j, 128)], identb[:],
                                is_transpose=True, start=(j == 0), stop=(j == jn - 1))
                        dst = ptT[:, c:c + jn, :].rearrange("p a b -> p (a b)")
                        src = tp[:, 0:jn * 128]
                        if ce in ("d", "m"):
                            nc.vector.tensor_copy(dst, src)
                        else:
                            nc.scalar.copy(dst, src)
                    if prev_pvst:
                        emit_pv_chunks(prev_pvst, g * pv_per_gap, (g + 1) * pv_per_gap)
                    c += jn
                    g += 1
                if prev_pvst:
                    emit_pv_chunks(prev_pvst, g * pv_per_gap, prev_nch)
                pvq = accp.tile([128, 65], f32, tag="acc", name="pvq")
                return (row, ptT, pvq)

            def emit_otT(sb_i, nout_t, ot_t):
                for hp in range(NPAIR):
                    for h in range(2):
                        otp = accp.tile([64, SB], bf16, tag="acc", name="otp")
                        for j in range(4):
                            nc.tensor.matmul(
                                otp[:, bass.ts(j, 128)],
                                nout_t[hp][:, h, j, :], identb[:],
                                is_transpose=True, start=(j == 0), stop=(j == 3))
                        nc.vector.tensor_copy(
                            ot_t[hp][bass.ds(h * 64, 64), :], otp[:])

            def emit_oproj_item(sb_i, mo, ot_t, ob):
                pso = accp.tile([128, SB], f32, tag="acc", name="pso")
                for fc in range(4):
                    nc.tensor.matmul(
                        pso[:], wo_b[:, fc, bass.ts(mo, 128)],
                        ot_t[fc][:, :], start=(fc == 0), stop=(fc == 3))
                if mo % 2 == 0:
                    nc.vector.tensor_copy(ob[:, mo, :], pso[:])
                else:
                    nc.scalar.copy(ob[:, mo, :], pso[:])

            def emit_out_dma(sb_i, ob):
                nc.sync.dma_start(
                    oT_d[:, bass.ts(sb_i, SB)].rearrange("(a p) b -> p a b", p=128),
                    ob[:])

            # ---------- main pipelined schedule ----------
            # Global row pipeline carried across seq-blocks; otT/o-proj for a
            # block are emitted as soon as its last row finishes, and o-proj
            # items weave between later rows as PE filler.
            pvst = None           # PV-state: row whose PV chunks go in next gaps
            pending = []          # rows awaiting transpose stage
            oproj_q = []          # pending o-proj filler items
            sb_state = {}         # sb_i -> dict(nout_t, ot_t, left, ob, emitted)

            def row_finished(row):
                hp, h, qi = row
                fsb = qi // 4
                st = sb_state[fsb]
                st["left"] -= 1
                if st["left"] == 0:
                    emit_otT(fsb, st["nout_t"], st["ot_t"])
                    ob = obp.tile([128, 16, SB], bf16, tag="ob", name="ob")
                    st["ob"] = ob
                    for mo in range(16):
                        oproj_q.append((fsb, mo))

            def pump_oproj(n):
                for _ in range(n):
                    if not oproj_q:
                        return
                    fsb, mo = oproj_q.pop(0)
                    st = sb_state[fsb]
                    emit_oproj_item(fsb, mo, st["ot_t"], st["ob"])
                    st["emitted"] += 1
                    if st["emitted"] == 16:
                        emit_out_dma(fsb, st["ob"])

            def pop_pending():
                nonlocal pvst
                prow, pp_t = pending.pop(0)
                pvst_new = emit_transpose(prow, pp_t, pvst)
                if pvst is not None:
                    fr = pvst[0]
                    emit_pv_finish(pvst, sb_state[fr[2] // 4]["nout_t"])
                    row_finished(fr)
                pvst = pvst_new

            for sb_i in range(NSB):
                emit_proj_pass(sb_i, 0)
                emit_proj_pass(sb_i, 1)
                sb_state[sb_i] = {
                    "nout_t": [nopool.tile([128, 2, 4, 64], bf16, tag=f"no{hp}",
                                           name=f"no{hp}") for hp in range(NPAIR)],
                    "ot_t": [otpool.tile([128, SB], bf16, tag=f"ot{hp}",
                                         name=f"ot{hp}") for hp in range(NPAIR)],
                    "left": 32, "ob": None, "emitted": 0,
                }
                rows = [(hp, h, sb_i * 4 + j)
                        for j in range(4) for hp in range(NPAIR) for h in range(2)]
                for ri, row in enumerate(rows):
                    p_t = emit_scores(row)
                    pending.append((row, p_t))
                    if ri % OPW == OPW - 1:
                        pump_oproj(1)
                    if len(pending) > ROW_LAG:
                        pop_pending()

            # drain the pipeline
            while pending:
                pop_pending()
            if pvst is not None:
                emit_pv_chunks(pvst, 0, pvst[0][2] + 1)
                fr = pvst[0]
                emit_pv_finish(pvst, sb_state[fr[2] // 4]["nout_t"])
                row_finished(fr)
                pvst = None
            pump_oproj(len(oproj_q) + 16)

    nc.compile()
    return nc


_PROG = None


def kernel(x, wq, wk, wv, wo):
    global _PROG
    if _PROG is None:
        _PROG = build_program()
    nc = _PROG

    twq = _ternarize(wq) / 8.0          # fold softmax scale into q
    twk = _ternarize(wk)
    twv = _ternarize(wv)
    two = _ternarize(wo)
    tri_np = (np.triu(np.ones((128, 128), np.float64), 1) * -1e30).astype(np.float32)

    xT = [np.ascontiguousarray(x[b].astype(np.float32).T) for b in range(BSZ)]
    in_maps = []
    for c in range(8):
        b, hq = c % 2, c // 2
        qcols = slice(hq * 512, (hq + 1) * 512)
        kvcols = slice(hq * 128, (hq + 1) * 128)
        in_maps.append({
            "xT": xT[b],
            "wq": np.ascontiguousarray(twq.T[:, qcols]).astype(np.float32),
            "wk": np.ascontiguousarray(twk.T[:, kvcols]).astype(np.float32),
            "wv": np.ascontiguousarray(twv.T[:, kvcols]).astype(np.float32),
            "wo": np.ascontiguousarray(two.T[hq * 512:(hq + 1) * 512, :]).astype(bf),
            "tri": tri_np,
        })

    res = run_bass_kernel_spmd(nc, in_maps, list(range(8)))

    out = np.zeros((BSZ, SEQ, DIM), np.float32)
    for c in range(8):
        b = c % 2
        out[b] += res.results[c]["oT"].astype(np.float32).T
    return out


# revision 40
# speedup vs baseline: 1.0507x; 1.0045x over previous
import sys
sys.path.insert(0, '/opt/trn_rl_repo')
import numpy as np
import ml_dtypes

import concourse.bass as bass
import concourse.tile as tile
from concourse import bacc, mybir
from concourse.bass_utils import run_bass_kernel_spmd
from concourse.masks import make_identity

DIM = 2048
BSZ, SEQ = 2, 2048
S = SEQ
THRESHOLD = 0.05
HPC = 8                      # q heads per core
KVPC = 2                     # kv heads per core
NPAIR = 4                    # q-head pairs per core
SB = 512
NSB = S // SB                # 4
NDC = DIM // 128             # 16 contraction chunks
NQT = S // 128               # 16 q tiles

f32 = mybir.dt.float32
f32r = mybir.dt.float32r
bf16 = mybir.dt.bfloat16
bf = ml_dtypes.bfloat16
EXP = mybir.ActivationFunctionType.Exp
AX = mybir.AxisListType.X
MAXOP = mybir.AluOpType.max
MINOP = mybir.AluOpType.min
ADDOP = mybir.AluOpType.add

import os
ROW_LAG = int(os.environ.get("K_ROW_LAG", "2"))
USE_DMA_T = os.environ.get("K_DMA_T", "0") == "1"
PTP_BUFS = int(os.environ.get("K_PTP", "4"))
PTT_BUFS = int(os.environ.get("K_PTT", "2"))
COPY_PAT = os.environ.get("K_COPY", "ad")   # per-group engine cycle: d=DVE a=ACT
OPW = int(os.environ.get("K_OPW", "1"))      # oproj weave: 1 item per OPW rows


def _ternarize(w):
    w = w.astype(np.float64)
    scale = max(np.abs(w).mean(), 1e-6)
    return np.where(w > THRESHOLD * scale, 1.0,
                    np.where(w < -THRESHOLD * scale, -1.0, 0.0))


def build_program():
    nc = bacc.Bacc(None, target_bir_lowering=False, debug=False)

    def din(name, shape, dt):
        return nc.dram_tensor(name, list(shape), dt, kind="ExternalInput").ap()

    xT_d = din("xT", (DIM, S), f32r)         # x[b].T fp32
    wq_d = din("wq", (DIM, 512), f32r)       # ternary(wq).T/8 cols (8 heads)
    wk_d = din("wk", (DIM, 128), f32r)
    wv_d = din("wv", (DIM, 128), f32r)
    wo_d = din("wo", (512, DIM), bf16)       # ternary(wo).T rows = core's feats
    tri_d = din("tri", (128, 128), f32)      # strictly-upper -1e30, else 0
    oT_d = nc.dram_tensor("oT", [DIM, S], bf16, kind="ExternalOutput").ap()

    with tile.TileContext(nc) as tc:
        with tc.tile_pool(name="persist", bufs=1) as pp, \
             tc.tile_pool(name="wts", bufs=1) as wp, \
             tc.tile_pool(name="xq", bufs=3) as xqp, \
             tc.tile_pool(name="vfp", bufs=2) as vfp, \
             tc.tile_pool(name="ptp", bufs=PTP_BUFS) as ptp, \
             tc.tile_pool(name="ptTp", bufs=PTT_BUFS) as ptTp, \
             tc.tile_pool(name="stp", bufs=6) as stp, \
             tc.tile_pool(name="otp", bufs=2) as otpool, \
             tc.tile_pool(name="nop", bufs=2) as nopool, \
             tc.tile_pool(name="obp", bufs=1) as obp, \
             tc.tile_pool(name="acc", bufs=3, space="PSUM") as accp, \
             tc.tile_pool(name="sps", bufs=5, space="PSUM") as sps:

            tri = pp.tile([128, 128], f32)
            nc.sync.dma_start(tri[:], tri_d[:])
            identb = pp.tile([128, 128], bf16)
            make_identity(nc, identb[:])
            # trz: cols 0:512 zeros, 512:640 = tri; slice [640-kw:640] puts the
            # causal mask on the last 128 cols of a kw-wide window.
            trz = pp.tile([128, 640], f32)
            nc.vector.memset(trz[:, 0:512], 0.0)
            nc.vector.tensor_copy(trz[:, 512:640], tri[:])

            qt = [pp.tile([128, S], f32r, tag=f"qt{m}", name=f"qt{m}") for m in range(NPAIR)]
            kk = [pp.tile([128, S], f32r, tag=f"kk{v}", name=f"kk{v}") for v in range(KVPC)]
            va = pp.tile([128, NDC, KVPC, 65], bf16)
            nc.vector.memset(va[:, :, :, 64:65], 1.0)

            # weights resident all run
            wq_f = wp.tile([128, NDC, 512], f32r)
            wk_f = wp.tile([128, NDC, 128], f32r)
            wv_f = wp.tile([128, NDC, 128], f32r)
            wo_b = wp.tile([128, 4, DIM], bf16)
            wstate = {"g": 0, "wo": False}

            def emit_w_dma_chunk():
                g = wstate["g"]
                if g < 4:
                    gs = bass.ds(g * 512, 512)
                    qs = bass.ds(g * 4, 4)
                    nc.sync.dma_start(
                        wq_f[:, qs, :],
                        wq_d[gs, :].rearrange("(a p) b -> p a b", p=128))
                    nc.sync.dma_start(
                        wk_f[:, qs, :],
                        wk_d[gs, :].rearrange("(a p) b -> p a b", p=128))
                    nc.sync.dma_start(
                        wv_f[:, qs, :],
                        wv_d[gs, :].rearrange("(a p) b -> p a b", p=128))
                    wstate["g"] = g + 1
                elif not wstate["wo"]:
                    nc.sync.dma_start(
                        wo_b[:], wo_d[:, :].rearrange("(a p) b -> p a b", p=128))
                    wstate["wo"] = True

            # ---------- emission helpers ----------
            def emit_x_dma(sb_i, g):
                """load dc quad g (4 chunks) of x for seq-block sb_i"""
                xt = xqp.tile([128, 4, SB], f32r, tag="x", name="xt")
                nc.sync.dma_start(
                    xt[:],
                    xT_d[g * 512:(g + 1) * 512, bass.ts(sb_i, SB)].rearrange(
                        "(a p) b -> p a b", p=128))
                return xt

            def emit_proj_pass(sb_i, which):
                """which=0: Q01+K ; which=1: Q23+V."""
                ssl = bass.ts(sb_i, SB)
                a0 = accp.tile([128, SB], f32, tag="acc", name="a0")
                a1 = accp.tile([128, SB], f32, tag="acc", name="a1")
                a2 = accp.tile([128, SB], f32, tag="acc", name="a2")
                xt = None
                for dc in range(NDC):
                    if dc % 4 == 0:
                        xt = emit_x_dma(sb_i, dc // 4)
                        emit_w_dma_chunk()
                    xr = xt[:, dc % 4, :]
                    st = (dc == 0)
                    sp = (dc == NDC - 1)
                    m0, m1 = (0, 1) if which == 0 else (2, 3)
                    nc.tensor.matmul(a0[:], wq_f[:, dc, bass.ts(m0, 128)],
                                     xr, start=st, stop=sp)
                    nc.tensor.matmul(a1[:], wq_f[:, dc, bass.ts(m1, 128)],
                                     xr, start=st, stop=sp)
                    wkv = wk_f if which == 0 else wv_f
                    nc.tensor.matmul(a2[:], wkv[:, dc, :],
                                     xr, start=st, stop=sp)
                # evacuations
                if which == 0:
                    nc.vector.tensor_copy(qt[0][:, ssl], a0[:])
                    nc.scalar.copy(qt[1][:, ssl], a1[:])
                    for v in range(KVPC):
                        nc.vector.tensor_copy(kk[v][0:64, ssl], a2[bass.ds(v * 64, 64), :])
                        nc.scalar.copy(kk[v][64:128, ssl], a2[bass.ds(v * 64, 64), :])
                else:
                    nc.vector.tensor_copy(qt[2][:, ssl], a0[:])
                    nc.scalar.copy(qt[3][:, ssl], a1[:])
                    vf = vfp.tile([128, SB], bf16, tag="vf")
                    nc.scalar.copy(vf[:], a2[:])
                    for j in range(4):
                        c = sb_i * 4 + j
                        vt = accp.tile([128, 128], bf16, tag="acc", name="vt")
                        nc.tensor.matmul(vt[:], vf[:, bass.ts(j, 128)], identb[:],
                                         is_transpose=True, start=True, stop=True)
                        nc.vector.tensor_copy(va[:, c, :, 0:64], vt[:])

            def emit_scores(row):
                """scores + max + exp for one attention row. Returns state."""
                hp, h, qi = row
                kv = hp // 2
                nk = qi // 4 + 1
                qsl = bass.ts(qi, 128)
                lhs_q = qt[hp][bass.ds(h * 64, 64), qsl]
                nmx = stp.tile([128, 4], f32, tag="nmx")
                sblk = []
                for kb in range(nk):
                    kw = 512 if kb < nk - 1 else 128 * (qi % 4 + 1)
                    s0 = sps.tile([128, SB], f32, tag="s", name=f"s{kb}")
                    sblk.append((s0, kw))
                    nc.tensor.matmul(
                        s0[:, 0:kw], lhs_q,
                        kk[kv][bass.ds(h * 64, 64), bass.ds(kb * 512, kw)],
                        start=True, stop=True, tile_position=(h * 64, 0))
                    if kb == nk - 1:
                        nc.vector.tensor_tensor(
                            s0[:, kw - 128:kw], s0[:, kw - 128:kw], tri[:], ADDOP)
                    nc.vector.tensor_reduce(
                        nmx[:, kb:kb + 1], s0[:, 0:kw], AX, MAXOP, negate=True)
                negmax = stp.tile([128, 1], f32, tag="ngm")
                nc.vector.tensor_reduce(negmax[:], nmx[:, 0:nk], AX, MINOP)
                p_t = ptp.tile([128, S], bf16, tag="p")
                for kb, (s0, kw) in enumerate(sblk):
                    nc.scalar.activation(
                        p_t[:, bass.ds(kb * 512, kw)], s0[:, 0:kw],
                        EXP, bias=negmax[:], scale=1.0)
                return p_t

            copy_rr = [0]

            def emit_pv_chunks(pvst, lo, hi):
                """emit PV matmul chunks [lo, hi) for a row's pv state"""
                row, ptT, pvq = pvst
                hp, h, qi = row
                kv = hp // 2
                nch = qi + 1
                for c in range(lo, min(hi, nch)):
                    nc.tensor.matmul(pvq[:], ptT[:, c, :], va[:, c, kv, :],
                                     start=(c == 0), stop=(c == nch - 1),
                                     skip_group_check=True)

            def emit_pv_finish(pvst, nout_t):
                row, ptT, pvq = pvst
                hp, h, qi = row
                rr = stp.tile([128, 1], f32, tag="rr")
                nc.vector.reciprocal(rr[:], pvq[:, 64:65])
                nc.vector.tensor_scalar_mul(nout_t[hp][:, h, qi % 4, :],
                                            pvq[:, 0:64], rr[:])

            def emit_transpose(row, p_t, prev_pvst):
                """transpose P chunks to ptT; weave prev row's PV between
                groups.  Returns (row, ptT, pvq) PV-state for this row."""
                hp, h, qi = row
                nch = qi + 1
                prev_nch = prev_pvst[0][2] + 1 if prev_pvst else 0
                ngroups = (nch + 3) // 4
                pv_per_gap = (prev_nch + ngroups - 1) // ngroups if prev_pvst else 0
                ptT = ptTp.tile([128, NQT, 128], bf16, tag="ptT")
                c = 0
                g = 0
                while c < nch:
                    jn = min(4, nch - c)
                    r = copy_rr[0] % len(COPY_PAT)
                    ce = COPY_PAT[r]
                    copy_rr[0] += 1
                    if ce == "x" or (USE_DMA_T and ce == "m"):
                        # DMA xbar transpose: no PSUM, no copy
                        for j in range(jn):
                            nc.sync.dma_start_transpose(
                                ptT[:, c + j, :], p_t[:, bass.ts(c + j, 128)])
                    else:
                        tp = accp.tile([128, SB], bf16, tag="acc", name="tp")
                        for j in range(jn):
                            nc.tensor.matmul(
                                tp[:, bass.ts(j, 128)],
                                p_t[:, bass.ts(c + j, 128)], identb[:],
                                is_transpose=True, start=(j == 0), stop=(j == jn - 1))
                        dst = ptT[:, c:c + jn, :].rearrange("p a b -> p (a b)")
                        src = tp[:, 0:jn * 128]
                        if ce in ("d", "m"):
                            nc.vector.tensor_copy(dst, src)
                        else:
                            nc.scalar.copy(dst, src)
                    if prev_pvst:
                        emit_pv_chunks(prev_pvst, g * pv_per_gap, (g + 1) * pv_per_gap)
                    c += jn
                    g += 1
                if prev_pvst:
                    emit_pv_chunks(prev_pvst, g * pv_per_gap, prev_nch)
                pvq = accp.tile([128, 65], f32, tag="acc", name="pvq")
                return (row, ptT, pvq)

            def emit_otT(sb_i, nout_t, ot_t):
                for hp in range(NPAIR):
                    for h in range(2):
                        otp = accp.tile([64, SB], bf16, tag="acc", name="otp")
                        for j in range(4):
                            nc.tensor.matmul(
                                otp[:, bass.ts(j, 128)],
                                nout_t[hp][:, h, j, :], identb[:],
                                is_transpose=True, start=(j == 0), stop=(j == 3))
                        nc.vector.tensor_copy(
                            ot_t[hp][bass.ds(h * 64, 64), :], otp[:])

            def emit_oproj_item(sb_i, mo, ot_t, ob):
                pso = accp.tile([128, SB], f32, tag="acc", name="pso")
                for fc in range(4):
                    nc.tensor.matmul(
                        pso[:], wo_b[:, fc, bass.ts(mo, 128)],
                        ot_t[fc][:, :], start=(fc == 0), stop=(fc == 3))
                if mo % 2 == 0:
                    nc.vector.tensor_copy(ob[:, mo, :], pso[:])
                else:
                    nc.scalar.copy(ob[:, mo, :], pso[:])

            def emit_out_dma(sb_i, ob):
                nc.sync.dma_start(
                    oT_d[:, bass.ts(sb_i, SB)].rearrange("(a p) b -> p a b", p=128),
                    ob[:])

            # ---------- main pipelined schedule ----------
            # Global row pipeline carried across seq-blocks; otT/o-proj for a
            # block are emitted as soon as its last row finishes, and o-proj
            # items weave between later rows as PE filler.
            pvst = None           # PV-state: row whose PV chunks go in next gaps
            pending = []          # rows awaiting transpose stage
            oproj_q = []          # pending o-proj filler items
            sb_state = {}         # sb_i -> dict(nout_t, ot_t, left, ob, emitted)

            def row_finished(row):
                hp, h, qi = row
                fsb = qi // 4
                st = sb_state[fsb]
                st["left"] -= 1
                if st["left"] == 0:
                    emit_otT(fsb, st["nout_t"], st["ot_t"])
                    ob = obp.tile([128, 16, SB], bf16, tag="ob", name="ob")
                    st["ob"] = ob
                    for mo in range(16):
                        oproj_q.append((fsb, mo))

            def pump_oproj(n):
                for _ in range(n):
                    if not oproj_q:
                        return
                    fsb, mo = oproj_q.pop(0)
                    st = sb_state[fsb]
                    emit_oproj_item(fsb, mo, st["ot_t"], st["ob"])
                    st["emitted"] += 1
                    if st["emitted"] == 16:
                        emit_out_dma(fsb, st["ob"])

            def pop_pending():
                nonlocal pvst
                prow, pp_t = pending.pop(0)
                pvst_new = emit_transpose(prow, pp_t, pvst)
                if pvst is not None:
                    fr = pvst[0]
                    emit_pv_finish(pvst, sb_state[fr[2] // 4]["nout_t"])
                    row_finished(fr)
                pvst = pvst_new

            for sb_i in range(NSB):
                emit_proj_pass(sb_i, 0)
                emit_proj_pass(sb_i, 1)
                sb_state[sb_i] = {
                    "nout_t": [nopool.tile([128, 2, 4, 64], bf16, tag=f"no{hp}",
                                           name=f"no{hp}") for hp in range(NPAIR)],
                    "ot_t": [otpool.tile([128, SB], bf16, tag=f"ot{hp}",
                                         name=f"ot{hp}") for hp in range(NPAIR)],
                    "left": 32, "ob": None, "emitted": 0,
                }
                rows = [(hp, h, sb_i * 4 + j)
                        for j in range(4) for hp in range(NPAIR) for h in range(2)]
                for ri, row in enumerate(rows):
                    p_t = emit_scores(row)
                    pending.append((row, p_t))
                    if ri % OPW == OPW - 1:
                        pump_oproj(1)
                    if len(pending) > ROW_LAG:
                        pop_pending()

            # drain the pipeline
            while pending:
                pop_pending()
            if pvst is not None:
                emit_pv_chunks(pvst, 0, pvst[0][2] + 1)
                fr = pvst[0]
                emit_pv_finish(pvst, sb_state[fr[2] // 4]["nout_t"])
                row_finished(fr)
                pvst = None
            pump_oproj(len(oproj_q) + 16)

    nc.compile()
    return nc


_PROG = None


def kernel(x, wq, wk, wv, wo):
    global _PROG
    if _PROG is None:
        _PROG = build_program()
    nc = _PROG

    twq = _ternarize(wq) / 8.0          # fold softmax scale into q
    twk = _ternarize(wk)
    twv = _ternarize(wv)
    two = _ternarize(wo)
    tri_np = (np.triu(np.ones((128, 128), np.float64), 1) * -1e30).astype(np.float32)

    xT = [np.ascontiguousarray(x[b].astype(np.float32).T) for b in range(BSZ)]
    in_maps = []
    for c in range(8):
        b, hq = c % 2, c // 2
        qcols = slice(hq * 512, (hq + 1) * 512)
        kvcols = slice(hq * 128, (hq + 1) * 128)
        in_maps.append({
            "xT": xT[b],
            "wq": np.ascontiguousarray(twq.T[:, qcols]).astype(np.float32),
            "wk": np.ascontiguousarray(twk.T[:, kvcols]).astype(np.float32),
            "wv": np.ascontiguousarray(twv.T[:, kvcols]).astype(np.float32),
            "wo": np.ascontiguousarray(two.T[hq * 512:(hq + 1) * 512, :]).astype(bf),
            "tri": tri_np,
        })

    res = run_bass_kernel_spmd(nc, in_maps, list(range(8)))

    out = np.zeros((BSZ, SEQ, DIM), np.float32)
    for c in range(8):
        b = c % 2
        out[b] += res.results[c]["oT"].astype(np.float32).T
    return out


# revision 41
# speedup vs baseline: 1.1216x; 1.0675x over previous
import sys
sys.path.insert(0, '/opt/trn_rl_repo')
import numpy as np
import ml_dtypes

import concourse.bass as bass
import concourse.tile as tile
from concourse import bacc, mybir
from concourse.bass_utils import run_bass_kernel_spmd
from concourse.masks import make_identity

DIM = 2048
BSZ, SEQ = 2, 2048
S = SEQ
THRESHOLD = 0.05
HPC = 8                      # q heads per core
KVPC = 2                     # kv heads per core
NPAIR = 4                    # q-head pairs per core
SB = 512
NSB = S // SB                # 4
NDC = DIM // 128             # 16 contraction chunks
NQT = S // 128               # 16 q tiles

f32 = mybir.dt.float32
f32r = mybir.dt.float32r
bf16 = mybir.dt.bfloat16
bf = ml_dtypes.bfloat16
EXP = mybir.ActivationFunctionType.Exp
AX = mybir.AxisListType.X
MAXOP = mybir.AluOpType.max
MINOP = mybir.AluOpType.min
ADDOP = mybir.AluOpType.add

import os
ROW_LAG = int(os.environ.get("K_ROW_LAG", "2"))
USE_DMA_T = os.environ.get("K_DMA_T", "0") == "1"
PTP_BUFS = int(os.environ.get("K_PTP", "4"))
PTT_BUFS = int(os.environ.get("K_PTT", "2"))
COPY_PAT = os.environ.get("K_COPY", "ad")   # per-group engine cycle: d=DVE a=ACT
OPW = int(os.environ.get("K_OPW", "1"))      # oproj weave: 1 item per OPW rows


def _ternarize(w):
    w = w.astype(np.float64)
    scale = max(np.abs(w).mean(), 1e-6)
    return np.where(w > THRESHOLD * scale, 1.0,
                    np.where(w < -THRESHOLD * scale, -1.0, 0.0))


def build_program():
    nc = bacc.Bacc(None, target_bir_lowering=False, debug=False)

    def din(name, shape, dt):
        return nc.dram_tensor(name, list(shape), dt, kind="ExternalInput").ap()

    xT_d = din("xT", (DIM, S), f32r)         # x[b].T fp32
    wq_d = din("wq", (DIM, 512), f32r)       # ternary(wq).T/8 cols (8 heads)
    wk_d = din("wk", (DIM, 128), f32r)
    wv_d = din("wv", (DIM, 128), f32r)
    wo_d = din("wo", (512, DIM), bf16)       # ternary(wo).T rows = core's feats
    tri_d = din("tri", (128, 128), f32)      # strictly-upper -1e30, else 0
    oT_d = nc.dram_tensor("oT", [DIM, S], bf16, kind="ExternalOutput").ap()

    with tile.TileContext(nc) as tc:
        with tc.tile_pool(name="persist", bufs=1) as pp, \
             tc.tile_pool(name="wts", bufs=1) as wp, \
             tc.tile_pool(name="xq", bufs=3) as xqp, \
             tc.tile_pool(name="vfp", bufs=2) as vfp, \
             tc.tile_pool(name="ptp", bufs=PTP_BUFS) as ptp, \
             tc.tile_pool(name="ptTp", bufs=PTT_BUFS) as ptTp, \
             tc.tile_pool(name="stp", bufs=6) as stp, \
             tc.tile_pool(name="otp", bufs=2) as otpool, \
             tc.tile_pool(name="nop", bufs=2) as nopool, \
             tc.tile_pool(name="obp", bufs=1) as obp, \
             tc.tile_pool(name="acc", bufs=3, space="PSUM") as accp, \
             tc.tile_pool(name="sps", bufs=5, space="PSUM") as sps:

            tri = pp.tile([128, 128], f32)
            nc.sync.dma_start(tri[:], tri_d[:])
            identb = pp.tile([128, 128], bf16)
            make_identity(nc, identb[:])
            # trz: cols 0:512 zeros, 512:640 = tri; slice [640-kw:640] puts the
            # causal mask on the last 128 cols of a kw-wide window.
            trz = pp.tile([128, 640], f32)
            nc.vector.memset(trz[:, 0:512], 0.0)
            nc.vector.tensor_copy(trz[:, 512:640], tri[:])
            trib = pp.tile([128, 128], bf16)
            nc.vector.tensor_copy(trib[:], tri[:])

            qt = [pp.tile([128, S], f32r, tag=f"qt{m}", name=f"qt{m}") for m in range(NPAIR)]
            kk = [pp.tile([128, S], f32r, tag=f"kk{v}", name=f"kk{v}") for v in range(KVPC)]
            va = pp.tile([128, NDC, KVPC, 65], bf16)
            nc.vector.memset(va[:, :, :, 64:65], 1.0)

            # weights resident all run
            wq_f = wp.tile([128, NDC, 512], f32r)
            wk_f = wp.tile([128, NDC, 128], f32r)
            wv_f = wp.tile([128, NDC, 128], f32r)
            wo_b = wp.tile([128, 4, DIM], bf16)
            wstate = {"g": 0, "wo": False}

            def emit_w_dma_chunk():
                g = wstate["g"]
                if g < 4:
                    gs = bass.ds(g * 512, 512)
                    qs = bass.ds(g * 4, 4)
                    nc.sync.dma_start(
                        wq_f[:, qs, :],
                        wq_d[gs, :].rearrange("(a p) b -> p a b", p=128))
                    nc.sync.dma_start(
                        wk_f[:, qs, :],
                        wk_d[gs, :].rearrange("(a p) b -> p a b", p=128))
                    nc.sync.dma_start(
                        wv_f[:, qs, :],
                        wv_d[gs, :].rearrange("(a p) b -> p a b", p=128))
                    wstate["g"] = g + 1
                elif not wstate["wo"]:
                    nc.sync.dma_start(
                        wo_b[:], wo_d[:, :].rearrange("(a p) b -> p a b", p=128))
                    wstate["wo"] = True

            # ---------- emission helpers ----------
            def emit_x_dma(sb_i, g):
                """load dc quad g (4 chunks) of x for seq-block sb_i"""
                xt = xqp.tile([128, 4, SB], f32r, tag="x", name="xt")
                nc.sync.dma_start(
                    xt[:],
                    xT_d[g * 512:(g + 1) * 512, bass.ts(sb_i, SB)].rearrange(
                        "(a p) b -> p a b", p=128))
                return xt

            def emit_proj_pass(sb_i, which):
                """which=0: Q01+K ; which=1: Q23+V."""
                ssl = bass.ts(sb_i, SB)
                a0 = accp.tile([128, SB], f32, tag="acc", name="a0")
                a1 = accp.tile([128, SB], f32, tag="acc", name="a1")
                a2 = accp.tile([128, SB], f32, tag="acc", name="a2")
                xt = None
                for dc in range(NDC):
                    if dc % 4 == 0:
                        xt = emit_x_dma(sb_i, dc // 4)
                        emit_w_dma_chunk()
                    xr = xt[:, dc % 4, :]
                    st = (dc == 0)
                    sp = (dc == NDC - 1)
                    m0, m1 = (0, 1) if which == 0 else (2, 3)
                    nc.tensor.matmul(a0[:], wq_f[:, dc, bass.ts(m0, 128)],
                                     xr, start=st, stop=sp)
                    nc.tensor.matmul(a1[:], wq_f[:, dc, bass.ts(m1, 128)],
                                     xr, start=st, stop=sp)
                    wkv = wk_f if which == 0 else wv_f
                    nc.tensor.matmul(a2[:], wkv[:, dc, :],
                                     xr, start=st, stop=sp)
                # evacuations
                if which == 0:
                    nc.vector.tensor_copy(qt[0][:, ssl], a0[:])
                    nc.scalar.copy(qt[1][:, ssl], a1[:])
                    for v in range(KVPC):
                        nc.vector.tensor_copy(kk[v][0:64, ssl], a2[bass.ds(v * 64, 64), :])
                        nc.scalar.copy(kk[v][64:128, ssl], a2[bass.ds(v * 64, 64), :])
                else:
                    nc.vector.tensor_copy(qt[2][:, ssl], a0[:])
                    nc.scalar.copy(qt[3][:, ssl], a1[:])
                    vf = vfp.tile([128, SB], bf16, tag="vf")
                    nc.scalar.copy(vf[:], a2[:])
                    for j in range(4):
                        c = sb_i * 4 + j
                        vt = accp.tile([128, 128], bf16, tag="acc", name="vt")
                        nc.tensor.matmul(vt[:], vf[:, bass.ts(j, 128)], identb[:],
                                         is_transpose=True, start=True, stop=True)
                        nc.vector.tensor_copy(va[:, c, :, 0:64], vt[:])

            def emit_scores(row):
                """scores + max + exp for one attention row. Returns state."""
                hp, h, qi = row
                kv = hp // 2
                nk = qi // 4 + 1
                qsl = bass.ts(qi, 128)
                lhs_q = qt[hp][bass.ds(h * 64, 64), qsl]
                nmx = stp.tile([128, 4], f32, tag="nmx")
                sblk = []
                for kb in range(nk):
                    kw = 512 if kb < nk - 1 else 128 * (qi % 4 + 1)
                    s0 = sps.tile([128, SB], f32, tag="s", name=f"s{kb}")
                    sblk.append((s0, kw))
                    diag = (kb == nk - 1)
                    nc.tensor.matmul(
                        s0[:, 0:kw], lhs_q,
                        kk[kv][bass.ds(h * 64, 64), bass.ds(kb * 512, kw)],
                        start=True, stop=not diag, tile_position=(h * 64, 0))
                    if diag:
                        # causal mask via PE: ident.T @ tri accumulates -1e30
                        # into the upper triangle of the diagonal 128-block
                        nc.tensor.matmul(
                            s0[:, kw - 128:kw], identb[:], trib[:],
                            start=False, stop=True, skip_group_check=True)
                    nc.vector.tensor_reduce(
                        nmx[:, kb:kb + 1], s0[:, 0:kw], AX, MAXOP, negate=True)
                if nk == 1:
                    negmax = nmx[:, 0:1]
                else:
                    negmax = stp.tile([128, 1], f32, tag="ngm")
                    nc.vector.tensor_reduce(negmax[:], nmx[:, 0:nk], AX, MINOP)
                p_t = ptp.tile([128, S], bf16, tag="p")
                for kb, (s0, kw) in enumerate(sblk):
                    nc.scalar.activation(
                        p_t[:, bass.ds(kb * 512, kw)], s0[:, 0:kw],
                        EXP, bias=negmax, scale=1.0)
                return p_t

            copy_rr = [0]

            def emit_pv_chunks(pvst, lo, hi):
                """emit PV matmul chunks [lo, hi) for a row's pv state"""
                row, ptT, pvq = pvst
                hp, h, qi = row
                kv = hp // 2
                nch = qi + 1
                for c in range(lo, min(hi, nch)):
                    nc.tensor.matmul(pvq[:], ptT[:, c, :], va[:, c, kv, :],
                                     start=(c == 0), stop=(c == nch - 1),
                                     skip_group_check=True)

            def emit_pv_finish(pvst, nout_t):
                row, ptT, pvq = pvst
                hp, h, qi = row
                rr = stp.tile([128, 1], f32, tag="rr")
                nc.vector.reciprocal(rr[:], pvq[:, 64:65])
                nc.vector.tensor_scalar_mul(nout_t[hp][:, h, qi % 4, :],
                                            pvq[:, 0:64], rr[:])

            def emit_transpose(row, p_t, prev_pvst):
                """transpose P chunks to ptT; weave prev row's PV between
                groups.  Returns (row, ptT, pvq) PV-state for this row."""
                hp, h, qi = row
                nch = qi + 1
                prev_nch = prev_pvst[0][2] + 1 if prev_pvst else 0
                ngroups = (nch + 3) // 4
                pv_per_gap = (prev_nch + ngroups - 1) // ngroups if prev_pvst else 0
                ptT = ptTp.tile([128, NQT, 128], bf16, tag="ptT")
                c = 0
                g = 0
                while c < nch:
                    jn = min(4, nch - c)
                    r = copy_rr[0] % len(COPY_PAT)
                    ce = COPY_PAT[r]
                    copy_rr[0] += 1
                    if ce == "x" or (USE_DMA_T and ce == "m"):
                        # DMA xbar transpose: no PSUM, no copy
                        for j in range(jn):
                            nc.sync.dma_start_transpose(
                                ptT[:, c + j, :], p_t[:, bass.ts(c + j, 128)])
                    else:
                        tp = accp.tile([128, SB], bf16, tag="acc", name="tp")
                        for j in range(jn):
                            nc.tensor.matmul(
                                tp[:, bass.ts(j, 128)],
                                p_t[:, bass.ts(c + j, 128)], identb[:],
                                is_transpose=True, start=(j == 0), stop=(j == jn - 1))
                        dst = ptT[:, c:c + jn, :].rearrange("p a b -> p (a b)")
                        src = tp[:, 0:jn * 128]
                        if ce in ("d", "m"):
                            nc.vector.tensor_copy(dst, src)
                        else:
                            nc.scalar.copy(dst, src)
                    if prev_pvst:
                        emit_pv_chunks(prev_pvst, g * pv_per_gap, (g + 1) * pv_per_gap)
                    c += jn
                    g += 1
                if prev_pvst:
                    emit_pv_chunks(prev_pvst, g * pv_per_gap, prev_nch)
                pvq = accp.tile([128, 65], f32, tag="acc", name="pvq")
                return (row, ptT, pvq)

            def emit_otT(sb_i, nout_t, ot_t):
                for hp in range(NPAIR):
                    for h in range(2):
                        otp = accp.tile([64, SB], bf16, tag="acc", name="otp")
                        for j in range(4):
                            nc.tensor.matmul(
                                otp[:, bass.ts(j, 128)],
                                nout_t[hp][:, h, j, :], identb[:],
                                is_transpose=True, start=(j == 0), stop=(j == 3))
                        nc.vector.tensor_copy(
                            ot_t[hp][bass.ds(h * 64, 64), :], otp[:])

            def emit_oproj_item(sb_i, mo, ot_t, ob):
                pso = accp.tile([128, SB], f32, tag="acc", name="pso")
                for fc in range(4):
                    nc.tensor.matmul(
                        pso[:], wo_b[:, fc, bass.ts(mo, 128)],
                        ot_t[fc][:, :], start=(fc == 0), stop=(fc == 3))
                if mo % 2 == 0:
                    nc.vector.tensor_copy(ob[:, mo, :], pso[:])
                else:
                    nc.scalar.copy(ob[:, mo, :], pso[:])

            def emit_out_dma(sb_i, ob):
                nc.sync.dma_start(
                    oT_d[:, bass.ts(sb_i, SB)].rearrange("(a p) b -> p a b", p=128),
                    ob[:])

            # ---------- main pipelined schedule ----------
            # Global row pipeline carried across seq-blocks; otT/o-proj for a
            # block are emitted as soon as its last row finishes, and o-proj
            # items weave between later rows as PE filler.
            pvst = None           # PV-state: row whose PV chunks go in next gaps
            pending = []          # rows awaiting transpose stage
            oproj_q = []          # pending o-proj filler items
            sb_state = {}         # sb_i -> dict(nout_t, ot_t, left, ob, emitted)

            def row_finished(row):
                hp, h, qi = row
                fsb = qi // 4
                st = sb_state[fsb]
                st["left"] -= 1
                if st["left"] == 0:
                    emit_otT(fsb, st["nout_t"], st["ot_t"])
                    ob = obp.tile([128, 16, SB], bf16, tag="ob", name="ob")
                    st["ob"] = ob
                    for mo in range(16):
                        oproj_q.append((fsb, mo))

            def pump_oproj(n):
                for _ in range(n):
                    if not oproj_q:
                        return
                    fsb, mo = oproj_q.pop(0)
                    st = sb_state[fsb]
                    emit_oproj_item(fsb, mo, st["ot_t"], st["ob"])
                    st["emitted"] += 1
                    if st["emitted"] == 16:
                        emit_out_dma(fsb, st["ob"])

            def pop_pending():
                nonlocal pvst
                prow, pp_t = pending.pop(0)
                pvst_new = emit_transpose(prow, pp_t, pvst)
                if pvst is not None:
                    fr = pvst[0]
                    emit_pv_finish(pvst, sb_state[fr[2] // 4]["nout_t"])
                    row_finished(fr)
                pvst = pvst_new

            for sb_i in range(NSB):
                emit_proj_pass(sb_i, 0)
                emit_proj_pass(sb_i, 1)
                sb_state[sb_i] = {
                    "nout_t": [nopool.tile([128, 2, 4, 64], bf16, tag=f"no{hp}",
                                           name=f"no{hp}") for hp in range(NPAIR)],
                    "ot_t": [otpool.tile([128, SB], bf16, tag=f"ot{hp}",
                                         name=f"ot{hp}") for hp in range(NPAIR)],
                    "left": 32, "ob": None, "emitted": 0,
                }
                rows = [(hp, h, sb_i * 4 + j)
                        for j in range(4) for hp in range(NPAIR) for h in range(2)]
                for ri, row in enumerate(rows):
                    p_t = emit_scores(row)
                    pending.append((row, p_t))
                    if ri % OPW == OPW - 1:
                        pump_oproj(1)
                    if len(pending) > ROW_LAG:
                        pop_pending()

            # drain the pipeline
            while pending:
                pop_pending()
            if pvst is not None:
                emit_pv_chunks(pvst, 0, pvst[0][2] + 1)
                fr = pvst[0]
                emit_pv_finish(pvst, sb_state[fr[2] // 4]["nout_t"])
                row_finished(fr)
                pvst = None
            pump_oproj(len(oproj_q) + 16)

    nc.compile()
    return nc


_PROG = None


def kernel(x, wq, wk, wv, wo):
    global _PROG
    if _PROG is None:
        _PROG = build_program()
    nc = _PROG

    twq = _ternarize(wq) / 8.0          # fold softmax scale into q
    twk = _ternarize(wk)
    twv = _ternarize(wv)
    two = _ternarize(wo)
    tri_np = (np.triu(np.ones((128, 128), np.float64), 1) * -1e30).astype(np.float32)

    xT = [np.ascontiguousarray(x[b].astype(np.float32).T) for b in range(BSZ)]
    in_maps = []
    for c in range(8):
        b, hq = c % 2, c // 2
        qcols = slice(hq * 512, (hq + 1) * 512)
        kvcols = slice(hq * 128, (hq + 1) * 128)
        in_maps.append({
            "xT": xT[b],
            "wq": np.ascontiguousarray(twq.T[:, qcols]).astype(np.float32),
            "wk": np.ascontiguousarray(twk.T[:, kvcols]).astype(np.float32),
            "wv": np.ascontiguousarray(twv.T[:, kvcols]).astype(np.float32),
            "wo": np.ascontiguousarray(two.T[hq * 512:(hq + 1) * 512, :]).astype(bf),
            "tri": tri_np,
        })

    res = run_bass_kernel_spmd(nc, in_maps, list(range(8)))

    out = np.zeros((BSZ, SEQ, DIM), np.float32)
    for c in range(8):
        b = c % 2
        out[b] += res.results[c]["oT"].astype(np.float32).T
    return out


# revision 42
# speedup vs baseline: 1.1236x; 1.0018x over previous
import sys
sys.path.insert(0, '/opt/trn_rl_repo')
import numpy as np
import ml_dtypes

import concourse.bass as bass
import concourse.tile as tile
from concourse import bacc, mybir
from concourse.bass_utils import run_bass_kernel_spmd
from concourse.masks import make_identity

DIM = 2048
BSZ, SEQ = 2, 2048
S = SEQ
THRESHOLD = 0.05
HPC = 8                      # q heads per core
KVPC = 2                     # kv heads per core
NPAIR = 4                    # q-head pairs per core
SB = 512
NSB = S // SB                # 4
NDC = DIM // 128             # 16 contraction chunks
NQT = S // 128               # 16 q tiles

f32 = mybir.dt.float32
f32r = mybir.dt.float32r
bf16 = mybir.dt.bfloat16
bf = ml_dtypes.bfloat16
EXP = mybir.ActivationFunctionType.Exp
AX = mybir.AxisListType.X
MAXOP = mybir.AluOpType.max
MINOP = mybir.AluOpType.min
ADDOP = mybir.AluOpType.add

import os
ROW_LAG = int(os.environ.get("K_ROW_LAG", "2"))
USE_DMA_T = os.environ.get("K_DMA_T", "0") == "1"
PTP_BUFS = int(os.environ.get("K_PTP", "5"))
PTT_BUFS = int(os.environ.get("K_PTT", "2"))
COPY_PAT = os.environ.get("K_COPY", "ad")   # per-group engine cycle: d=DVE a=ACT
OPW = int(os.environ.get("K_OPW", "1"))      # oproj weave: 1 item per OPW rows


def _ternarize(w):
    w = w.astype(np.float64)
    scale = max(np.abs(w).mean(), 1e-6)
    return np.where(w > THRESHOLD * scale, 1.0,
                    np.where(w < -THRESHOLD * scale, -1.0, 0.0))


def build_program():
    nc = bacc.Bacc(None, target_bir_lowering=False, debug=False)

    def din(name, shape, dt):
        return nc.dram_tensor(name, list(shape), dt, kind="ExternalInput").ap()

    xT_d = din("xT", (DIM, S), f32r)         # x[b].T fp32
    wq_d = din("wq", (DIM, 512), f32r)       # ternary(wq).T/8 cols (8 heads)
    wk_d = din("wk", (DIM, 128), f32r)
    wv_d = din("wv", (DIM, 128), f32r)
    wo_d = din("wo", (512, DIM), bf16)       # ternary(wo).T rows = core's feats
    tri_d = din("tri", (128, 128), f32)      # strictly-upper -1e30, else 0
    oT_d = nc.dram_tensor("oT", [DIM, S], bf16, kind="ExternalOutput").ap()

    with tile.TileContext(nc) as tc:
        with tc.tile_pool(name="persist", bufs=1) as pp, \
             tc.tile_pool(name="wts", bufs=1) as wp, \
             tc.tile_pool(name="xq", bufs=3) as xqp, \
             tc.tile_pool(name="vfp", bufs=2) as vfp, \
             tc.tile_pool(name="ptp", bufs=PTP_BUFS) as ptp, \
             tc.tile_pool(name="ptTp", bufs=PTT_BUFS) as ptTp, \
             tc.tile_pool(name="stp", bufs=6) as stp, \
             tc.tile_pool(name="otp", bufs=2) as otpool, \
             tc.tile_pool(name="nop", bufs=2) as nopool, \
             tc.tile_pool(name="obp", bufs=1) as obp, \
             tc.tile_pool(name="acc", bufs=3, space="PSUM") as accp, \
             tc.tile_pool(name="sps", bufs=5, space="PSUM") as sps:

            tri = pp.tile([128, 128], f32)
            nc.sync.dma_start(tri[:], tri_d[:])
            identb = pp.tile([128, 128], bf16)
            make_identity(nc, identb[:])
            # trz: cols 0:512 zeros, 512:640 = tri; slice [640-kw:640] puts the
            # causal mask on the last 128 cols of a kw-wide window.
            trz = pp.tile([128, 640], f32)
            nc.vector.memset(trz[:, 0:512], 0.0)
            nc.vector.tensor_copy(trz[:, 512:640], tri[:])
            trib = pp.tile([128, 128], bf16)
            nc.vector.tensor_copy(trib[:], tri[:])

            qt = [pp.tile([128, S], f32r, tag=f"qt{m}", name=f"qt{m}") for m in range(NPAIR)]
            kk = [pp.tile([128, S], f32r, tag=f"kk{v}", name=f"kk{v}") for v in range(KVPC)]
            va = pp.tile([128, NDC, KVPC, 65], bf16)
            nc.vector.memset(va[:, :, :, 64:65], 1.0)

            # weights resident all run
            wq_f = wp.tile([128, NDC, 512], f32r)
            wk_f = wp.tile([128, NDC, 128], f32r)
            wv_f = wp.tile([128, NDC, 128], f32r)
            wo_b = wp.tile([128, 4, DIM], bf16)
            wstate = {"g": 0, "wo": False}

            def emit_w_dma_chunk():
                g = wstate["g"]
                if g < 4:
                    gs = bass.ds(g * 512, 512)
                    qs = bass.ds(g * 4, 4)
                    nc.sync.dma_start(
                        wq_f[:, qs, :],
                        wq_d[gs, :].rearrange("(a p) b -> p a b", p=128))
                    nc.sync.dma_start(
                        wk_f[:, qs, :],
                        wk_d[gs, :].rearrange("(a p) b -> p a b", p=128))
                    nc.sync.dma_start(
                        wv_f[:, qs, :],
                        wv_d[gs, :].rearrange("(a p) b -> p a b", p=128))
                    wstate["g"] = g + 1
                elif not wstate["wo"]:
                    nc.sync.dma_start(
                        wo_b[:], wo_d[:, :].rearrange("(a p) b -> p a b", p=128))
                    wstate["wo"] = True

            # ---------- emission helpers ----------
            def emit_x_dma(sb_i, g):
                """load dc quad g (4 chunks) of x for seq-block sb_i"""
                xt = xqp.tile([128, 4, SB], f32r, tag="x", name="xt")
                nc.sync.dma_start(
                    xt[:],
                    xT_d[g * 512:(g + 1) * 512, bass.ts(sb_i, SB)].rearrange(
                        "(a p) b -> p a b", p=128))
                return xt

            def emit_proj_pass(sb_i, which):
                """which=0: Q01+K ; which=1: Q23+V."""
                ssl = bass.ts(sb_i, SB)
                a0 = accp.tile([128, SB], f32, tag="acc", name="a0")
                a1 = accp.tile([128, SB], f32, tag="acc", name="a1")
                a2 = accp.tile([128, SB], f32, tag="acc", name="a2")
                xt = None
                for dc in range(NDC):
                    if dc % 4 == 0:
                        xt = emit_x_dma(sb_i, dc // 4)
                        emit_w_dma_chunk()
                    xr = xt[:, dc % 4, :]
                    st = (dc == 0)
                    sp = (dc == NDC - 1)
                    m0, m1 = (0, 1) if which == 0 else (2, 3)
                    nc.tensor.matmul(a0[:], wq_f[:, dc, bass.ts(m0, 128)],
                                     xr, start=st, stop=sp)
                    nc.tensor.matmul(a1[:], wq_f[:, dc, bass.ts(m1, 128)],
                                     xr, start=st, stop=sp)
                    wkv = wk_f if which == 0 else wv_f
                    nc.tensor.matmul(a2[:], wkv[:, dc, :],
                                     xr, start=st, stop=sp)
                # evacuations
                if which == 0:
                    nc.vector.tensor_copy(qt[0][:, ssl], a0[:])
                    nc.scalar.copy(qt[1][:, ssl], a1[:])
                    for v in range(KVPC):
                        nc.vector.tensor_copy(kk[v][0:64, ssl], a2[bass.ds(v * 64, 64), :])
                        nc.scalar.copy(kk[v][64:128, ssl], a2[bass.ds(v * 64, 64), :])
                else:
                    nc.vector.tensor_copy(qt[2][:, ssl], a0[:])
                    nc.scalar.copy(qt[3][:, ssl], a1[:])
                    vf = vfp.tile([128, SB], bf16, tag="vf")
                    nc.scalar.copy(vf[:], a2[:])
                    for j in range(4):
                        c = sb_i * 4 + j
                        vt = accp.tile([128, 128], bf16, tag="acc", name="vt")
                        nc.tensor.matmul(vt[:], vf[:, bass.ts(j, 128)], identb[:],
                                         is_transpose=True, start=True, stop=True)
                        nc.vector.tensor_copy(va[:, c, :, 0:64], vt[:])

            def emit_scores(row):
                """scores + max + exp for one attention row. Returns state."""
                hp, h, qi = row
                kv = hp // 2
                nk = qi // 4 + 1
                qsl = bass.ts(qi, 128)
                lhs_q = qt[hp][bass.ds(h * 64, 64), qsl]
                nmx = stp.tile([128, 4], f32, tag="nmx")
                sblk = []
                for kb in range(nk):
                    kw = 512 if kb < nk - 1 else 128 * (qi % 4 + 1)
                    s0 = sps.tile([128, SB], f32, tag="s", name=f"s{kb}")
                    sblk.append((s0, kw))
                    diag = (kb == nk - 1)
                    nc.tensor.matmul(
                        s0[:, 0:kw], lhs_q,
                        kk[kv][bass.ds(h * 64, 64), bass.ds(kb * 512, kw)],
                        start=True, stop=not diag, tile_position=(h * 64, 0))
                    if diag:
                        # causal mask via PE: ident.T @ tri accumulates -1e30
                        # into the upper triangle of the diagonal 128-block
                        nc.tensor.matmul(
                            s0[:, kw - 128:kw], identb[:], trib[:],
                            start=False, stop=True, skip_group_check=True)
                    nc.vector.tensor_reduce(
                        nmx[:, kb:kb + 1], s0[:, 0:kw], AX, MAXOP, negate=True)
                if nk == 1:
                    negmax = nmx[:, 0:1]
                else:
                    negmax = stp.tile([128, 1], f32, tag="ngm")
                    nc.vector.tensor_reduce(negmax[:], nmx[:, 0:nk], AX, MINOP)
                p_t = ptp.tile([128, S], bf16, tag="p")
                for kb, (s0, kw) in enumerate(sblk):
                    nc.scalar.activation(
                        p_t[:, bass.ds(kb * 512, kw)], s0[:, 0:kw],
                        EXP, bias=negmax, scale=1.0)
                return p_t

            copy_rr = [0]

            def emit_pv_chunks(pvst, lo, hi):
                """emit PV matmul chunks [lo, hi) for a row's pv state"""
                row, ptT, pvq = pvst
                hp, h, qi = row
                kv = hp // 2
                nch = qi + 1
                for c in range(lo, min(hi, nch)):
                    nc.tensor.matmul(pvq[:], ptT[:, c, :], va[:, c, kv, :],
                                     start=(c == 0), stop=(c == nch - 1),
                                     skip_group_check=True)

            def emit_pv_finish(pvst, nout_t):
                row, ptT, pvq = pvst
                hp, h, qi = row
                rr = stp.tile([128, 1], f32, tag="rr")
                nc.vector.reciprocal(rr[:], pvq[:, 64:65])
                nc.vector.tensor_scalar_mul(nout_t[hp][:, h, qi % 4, :],
                                            pvq[:, 0:64], rr[:])

            def emit_transpose(row, p_t, prev_pvst):
                """transpose P chunks to ptT; weave prev row's PV between
                groups.  Returns (row, ptT, pvq) PV-state for this row."""
                hp, h, qi = row
                nch = qi + 1
                prev_nch = prev_pvst[0][2] + 1 if prev_pvst else 0
                ngroups = (nch + 3) // 4
                pv_per_gap = (prev_nch + ngroups - 1) // ngroups if prev_pvst else 0
                ptT = ptTp.tile([128, NQT, 128], bf16, tag="ptT")
                c = 0
                g = 0
                while c < nch:
                    jn = min(4, nch - c)
                    r = copy_rr[0] % len(COPY_PAT)
                    ce = COPY_PAT[r]
                    copy_rr[0] += 1
                    if ce == "x" or (USE_DMA_T and ce == "m"):
                        # DMA xbar transpose: no PSUM, no copy
                        for j in range(jn):
                            nc.sync.dma_start_transpose(
                                ptT[:, c + j, :], p_t[:, bass.ts(c + j, 128)])
                    else:
                        tp = accp.tile([128, SB], bf16, tag="acc", name="tp")
                        for j in range(jn):
                            nc.tensor.matmul(
                                tp[:, bass.ts(j, 128)],
                                p_t[:, bass.ts(c + j, 128)], identb[:],
                                is_transpose=True, start=(j == 0), stop=(j == jn - 1))
                        dst = ptT[:, c:c + jn, :].rearrange("p a b -> p (a b)")
                        src = tp[:, 0:jn * 128]
                        if ce in ("d", "m"):
                            nc.vector.tensor_copy(dst, src)
                        else:
                            nc.scalar.copy(dst, src)
                    if prev_pvst:
                        emit_pv_chunks(prev_pvst, g * pv_per_gap, (g + 1) * pv_per_gap)
                    c += jn
                    g += 1
                if prev_pvst:
                    emit_pv_chunks(prev_pvst, g * pv_per_gap, prev_nch)
                pvq = accp.tile([128, 65], f32, tag="acc", name="pvq")
                return (row, ptT, pvq)

            def emit_otT(sb_i, nout_t, ot_t):
                for hp in range(NPAIR):
                    for h in range(2):
                        otp = accp.tile([64, SB], bf16, tag="acc", name="otp")
                        for j in range(4):
                            nc.tensor.matmul(
                                otp[:, bass.ts(j, 128)],
                                nout_t[hp][:, h, j, :], identb[:],
                                is_transpose=True, start=(j == 0), stop=(j == 3))
                        nc.vector.tensor_copy(
                            ot_t[hp][bass.ds(h * 64, 64), :], otp[:])

            def emit_oproj_item(sb_i, mo, ot_t, ob):
                pso = accp.tile([128, SB], f32, tag="acc", name="pso")
                for fc in range(4):
                    nc.tensor.matmul(
                        pso[:], wo_b[:, fc, bass.ts(mo, 128)],
                        ot_t[fc][:, :], start=(fc == 0), stop=(fc == 3))
                if mo % 2 == 0:
                    nc.vector.tensor_copy(ob[:, mo, :], pso[:])
                else:
                    nc.scalar.copy(ob[:, mo, :], pso[:])

            def emit_out_dma(sb_i, ob):
                nc.sync.dma_start(
                    oT_d[:, bass.ts(sb_i, SB)].rearrange("(a p) b -> p a b", p=128),
                    ob[:])

            # ---------- main pipelined schedule ----------
            # Global row pipeline carried across seq-blocks; otT/o-proj for a
            # block are emitted as soon as its last row finishes, and o-proj
            # items weave between later rows as PE filler.
            pvst = None           # PV-state: row whose PV chunks go in next gaps
            pending = []          # rows awaiting transpose stage
            oproj_q = []          # pending o-proj filler items
            sb_state = {}         # sb_i -> dict(nout_t, ot_t, left, ob, emitted)

            def row_finished(row):
                hp, h, qi = row
                fsb = qi // 4
                st = sb_state[fsb]
                st["left"] -= 1
                if st["left"] == 0:
                    emit_otT(fsb, st["nout_t"], st["ot_t"])
                    ob = obp.tile([128, 16, SB], bf16, tag="ob", name="ob")
                    st["ob"] = ob
                    for mo in range(16):
                        oproj_q.append((fsb, mo))

            def pump_oproj(n):
                for _ in range(n):
                    if not oproj_q:
                        return
                    fsb, mo = oproj_q.pop(0)
                    st = sb_state[fsb]
                    emit_oproj_item(fsb, mo, st["ot_t"], st["ob"])
                    st["emitted"] += 1
                    if st["emitted"] == 16:
                        emit_out_dma(fsb, st["ob"])

            def pop_pending():
                nonlocal pvst
                prow, pp_t = pending.pop(0)
                pvst_new = emit_transpose(prow, pp_t, pvst)
                if pvst is not None:
                    fr = pvst[0]
                    emit_pv_finish(pvst, sb_state[fr[2] // 4]["nout_t"])
                    row_finished(fr)
                pvst = pvst_new

            for sb_i in range(NSB):
                emit_proj_pass(sb_i, 0)
                emit_proj_pass(sb_i, 1)
                sb_state[sb_i] = {
                    "nout_t": [nopool.tile([128, 2, 4, 64], bf16, tag=f"no{hp}",
                                           name=f"no{hp}") for hp in range(NPAIR)],
                    "ot_t": [otpool.tile([128, SB], bf16, tag=f"ot{hp}",
                                         name=f"ot{hp}") for hp in range(NPAIR)],
                    "left": 32, "ob": None, "emitted": 0,
                }
                rows = [(hp, h, sb_i * 4 + j)
                        for j in range(4) for hp in range(NPAIR) for h in range(2)]
                for ri, row in enumerate(rows):
                    p_t = emit_scores(row)
                    pending.append((row, p_t))
                    if ri % OPW == OPW - 1:
                        pump_oproj(1)
                    if len(pending) > ROW_LAG:
                        pop_pending()

            # drain the pipeline
            while pending:
                pop_pending()
            if pvst is not None:
                emit_pv_chunks(pvst, 0, pvst[0][2] + 1)
                fr = pvst[0]
                emit_pv_finish(pvst, sb_state[fr[2] // 4]["nout_t"])
                row_finished(fr)
                pvst = None
            pump_oproj(len(oproj_q) + 16)

    nc.compile()
    return nc


_PROG = None


def kernel(x, wq, wk, wv, wo):
    global _PROG
    if _PROG is None:
        _PROG = build_program()
    nc = _PROG

    twq = _ternarize(wq) / 8.0          # fold softmax scale into q
    twk = _ternarize(wk)
    twv = _ternarize(wv)
    two = _ternarize(wo)
    tri_np = (np.triu(np.ones((128, 128), np.float64), 1) * -1e30).astype(np.float32)

    xT = [np.ascontiguousarray(x[b].astype(np.float32).T) for b in range(BSZ)]
    in_maps = []
    for c in range(8):
        b, hq = c % 2, c // 2
        qcols = slice(hq * 512, (hq + 1) * 512)
        kvcols = slice(hq * 128, (hq + 1) * 128)
        in_maps.append({
            "xT": xT[b],
            "wq": np.ascontiguousarray(twq.T[:, qcols]).astype(np.float32),
            "wk": np.ascontiguousarray(twk.T[:, kvcols]).astype(np.float32),
            "wv": np.ascontiguousarray(twv.T[:, kvcols]).astype(np.float32),
            "wo": np.ascontiguousarray(two.T[hq * 512:(hq + 1) * 512, :]).astype(bf),
            "tri": tri_np,
        })

    res = run_bass_kernel_spmd(nc, in_maps, list(range(8)))

    out = np.zeros((BSZ, SEQ, DIM), np.float32)
    for c in range(8):
        b = c % 2
        out[b] += res.results[c]["oT"].astype(np.float32).T
    return out
